# revision 8
# baseline (speedup 1.0000x reference)
"""GCN aggregator kernel for Trainium2 (Bass/Tile), 8-core data-parallel.

Computes: out = relu(((sum_g x[:,g,:]) / (K+1)) @ W + b), x = [neigh;self]
Sharding: nodes (N) split evenly across 8 NeuronCores; W replicated. b is
zeros per the problem spec and is dropped on device.

The kernel is HBM-bandwidth bound (~359 GB/s/NC, the 8-core share of the
chip's HBM). The rel-err budget (2e-2) allows fp8: most of the neighbor
stream is cast to fp8_e3m4 (4 mantissa bits) on the host with
error-feedback rounding along the group axis (residual carried
group-to-group, absorbed by the final bf16 groups), which keeps
end-to-end max rel err at the bf16-baseline level (~4e-3) while cutting
HBM traffic ~45%.

fp8 costs compute: DVE runs 1-byte ops at 1x (no fp8 packing on TRN2),
so the 26-group reduction no longer fits on DVE alone in the shortened
DMA period. The reduction is split across engines per 256-node tile
(2 half-tiles of 128 nodes; a group's two halves are adjacent per
partition, so every operand stays a flat contiguous slice - strided APs
defeat DVE 2x mode, and 22.5KB partition lines keep the DMA efficient):
  - K_BF groups stay bf16 in the stream (DVE adds them at 2x)
  - Q_POOL fp8 groups pair-add + self-merge on GPSIMD (idle otherwise);
    its single partial merges last in DVE's tree (no DVE stall)
  - P_PE fp8 groups are node-major matmul-accumulated (lhsT=I) per half
    into a PSUM f32 tile by the PE (warm under sustained load); DVE's
    merged partial is injected there as one more matmul
  - DVE pair-adds the rest fp8->bf16 + folds the partial tree
Then per half: ACT scaled-copy (1/26) PSUM->bf16, PE transposes, ACT
copy to SBUF, PE GEMM vs bf16 W (f32 PSUM), ACT relu, bf16 store.

The emission is software-pipelined in 3 stages (load+DVE/Pool reduce @
tile i, PE accumulate @ i-1, transpose+GEMM+relu+store @ i-2) so the
in-order PE stream never blocks on the per-tile PE<->ACT ping-pong and
each PSUM accumulation group stays contiguous in the PE stream.

Host: fp8/bf16 packing + error feedback in numpy; nodes re-tiled as
[tile, p, g, half, 512]; W pre-rearranged to [p, c, o]; bf16 output
unpacked and upcast to f32.
"""

import os
import sys

import numpy as np
import ml_dtypes

for _p in ("/opt/trn_rl_repo", "/root/.axon_site/_ro/trn_rl_repo"):
    if os.path.isdir(_p) and _p not in sys.path:
        sys.path.insert(0, _p)

import concourse.bass as bass
import concourse.tile as tile
from concourse import bacc, mybir
from concourse.masks import make_identity

N, K, D, O = 16384, 25, 512, 1024
G = K + 1  # neigh groups + self
N_CORES = 8
P = 128  # partition count
H = 2  # node half-tiles per tile (256 nodes/tile)
D2 = H * D  # per-group bytes-per-partition unit (both halves)
INV = 1.0 / (K + 1)
FP = mybir.dt.float32
BF = mybir.dt.bfloat16
F8 = mybir.dt.float8e3
NP_BF = ml_dtypes.bfloat16
NP_F8 = ml_dtypes.float8_e3m4

# reduction split (groups): GPSIMD | PE | DVE fp8 | DVE bf16
Q_POOL = 6
P_PE = 10
K_BF = 4
R_DVE = G - Q_POOL - P_PE - K_BF  # 6
G8 = G - K_BF  # fp8 groups in the stream (22)
PEB = Q_POOL + P_PE  # end of PE slab (16)
# partial slots (D2-wide): 0-2 DVE fp8, 3-4 DVE bf16, 5 GPSIMD
NPART = R_DVE // 2 + K_BF // 2 + 1  # 6


def build_nc(n_nodes: int, neigh_bufs: int = 4) -> bass.Bass:
    """Build the per-core Bass program for a shard of `n_nodes` nodes."""
    assert n_nodes % (P * H) == 0
    nt = n_nodes // (P * H)
    rows = n_nodes // H  # DRAM rows (one per partition-line)

    nc = bacc.Bacc("TRN2", target_bir_lowering=False, debug=False)
    pk8_h = nc.dram_tensor("pk8", [rows, G8 * D2], F8, kind="ExternalInput")
    pkb_h = nc.dram_tensor("pkb", [rows, K_BF * D2], BF, kind="ExternalInput")
    # W pre-rearranged on host to [p, c, o] (row p = partition line of every
    # d-chunk's rhs) so the device load is contiguous 8KB rows
    w_h = nc.dram_tensor("W", [P, (D // P) * O], BF, kind="ExternalInput")
    out_h = nc.dram_tensor("out", [rows, H * O], BF, kind="ExternalOutput")

    n_dc = D // P  # d-chunks for transposes / GEMM contraction
    n_oh = O // 512

    def gg(t, a, b):  # flat slice of D2-wide group units [a, b)
        return t[:, a * D2 : b * D2]

    with tile.TileContext(nc) as tc:
        with (
            tc.tile_pool(name="const", bufs=1) as const_pool,
            tc.tile_pool(name="neigh", bufs=neigh_bufs) as neigh_pool,
            tc.tile_pool(name="parts", bufs=3) as parts_pool,
            tc.tile_pool(name="pp", bufs=3) as pp_pool,
            tc.tile_pool(name="small", bufs=3) as small_pool,
            tc.tile_pool(name="outp", bufs=3) as out_pool,
            tc.tile_pool(name="ps_a", bufs=2, space="PSUM") as ps_a_pool,
            tc.tile_pool(name="ps_t", bufs=2, space="PSUM") as ps_t_pool,
            tc.tile_pool(name="ps_o", bufs=2, space="PSUM") as ps_o_pool,
        ):
            w_sb = const_pool.tile([P, n_dc * O], BF)
            ident = const_pool.tile([P, P], BF)
            make_identity(nc, ident)
            # W rides the scalar hwdge queue, parallel to the neigh stream
            nc.scalar.dma_start(w_sb, w_h[:, :])

            nh8s, parts_l, psA_l, means_l = {}, {}, {}, {}

            def stage_load(i):
                nh8 = neigh_pool.tile([P, G8 * D2], F8, tag="nh8", name="nh8")
                r = bass.ts(i, P)
                # consumer-ordered slabs: pool | pe | dve-fp8 | dve-bf16
                nc.sync.dma_start(gg(nh8, 0, Q_POOL), pk8_h[r, : Q_POOL * D2])
                nc.sync.dma_start(
                    gg(nh8, Q_POOL, PEB), pk8_h[r, Q_POOL * D2 : PEB * D2]
                )
                nc.sync.dma_start(gg(nh8, PEB, G8), pk8_h[r, PEB * D2 :])
                nhb = neigh_pool.tile([P, K_BF * D2], BF, tag="nhb", name="nhb")
                nc.sync.dma_start(nhb, pkb_h[r, :])
                nh8s[i] = (nh8, nhb)

            def stage_reduce(i):
                nh8, nhb = nh8s[i]
                parts = parts_pool.tile([P, NPART * D2], BF, tag="pt", name="pt")
                parts_l[i] = parts
                # GPSIMD: groups [0,6) pair-added then self-merged -> slot 5
                pp = pp_pool.tile([P, 3 * D2], BF, tag="pp", name="pp")
                nc.gpsimd.tensor_add(pp, gg(nh8, 0, 3), gg(nh8, 3, 6))
                nc.gpsimd.tensor_add(gg(pp, 0, 1), gg(pp, 0, 1), gg(pp, 1, 2))
                nc.gpsimd.tensor_add(gg(parts, 5, 6), gg(pp, 0, 1), gg(pp, 2, 3))
                # DVE: fp8 pairs -> slots 0-2 (1x), bf16 pairs -> 3-4 (2x),
                # tree with the Pool-dependent merge last
                nc.vector.tensor_add(
                    gg(parts, 0, 3), gg(nh8, PEB, PEB + 3), gg(nh8, PEB + 3, G8)
                )
                nc.vector.tensor_add(
                    gg(parts, 3, 5), gg(nhb, 0, 2), gg(nhb, 2, 4)
                )
                nc.vector.tensor_add(gg(parts, 0, 2), gg(parts, 0, 2), gg(parts, 2, 4))
                nc.vector.tensor_add(gg(parts, 0, 1), gg(parts, 0, 1), gg(parts, 1, 2))
                nc.vector.tensor_add(gg(parts, 0, 1), gg(parts, 0, 1), gg(parts, 4, 5))
                nc.vector.tensor_add(gg(parts, 0, 1), gg(parts, 0, 1), gg(parts, 5, 6))

            def stage_pe(i):
                # PE: per half, node-major accumulate raw fp8 groups + DVE's
                # merged partial (lhsT=I) in one contiguous PSUM group; ACT
                # scaled-copies (1/26) each half into bf16 means
                nh8, _ = nh8s[i]
                parts = parts_l.pop(i)
                means = small_pool.tile([P, D2], BF, tag="mn", name="mn")
                means_l[i] = means
                for h in range(H):
                    psA = ps_a_pool.tile([P, D], FP, tag="psA", name="psA")
                    for j in range(P_PE):
                        g = Q_POOL + j
                        nc.tensor.matmul(
                            psA,
                            lhsT=ident,
                            rhs=nh8[:, g * D2 + h * D : g * D2 + h * D + D],
                            start=(j == 0),
                            stop=False,
                        )
                    nc.tensor.matmul(
                        psA,
                        lhsT=ident,
                        rhs=parts[:, h * D : (h + 1) * D],
                        start=False,
                        stop=True,
                    )
                    nc.scalar.activation(
                        means[:, h * D : (h + 1) * D],
                        psA,
                        mybir.ActivationFunctionType.Copy,
                        scale=INV,
                    )

            def stage_gemm(i):
                means = means_l.pop(i)
                sumT = small_pool.tile([P, D2], BF, tag="tsb", name="tsb")
                tps = ps_t_pool.tile([P, D2], BF, tag="tps", name="tps")
                for h in range(H):
                    for c in range(n_dc):
                        s = h * D + c * P
                        nc.tensor.transpose(
                            tps[:, s : s + P], means[:, s : s + P], ident
                        )
                nc.scalar.activation(sumT, tps, mybir.ActivationFunctionType.Copy)
                out_sb = out_pool.tile([P, H * O], BF)
                for h in range(H):
                    # tags shared across halves: bufs=2 holds both halves
                    # in flight within 4 PSUM banks
                    out_pss = [
                        ps_o_pool.tile([P, 512], FP, tag=f"ops{oh}", name=f"ops{oh}")
                        for oh in range(n_oh)
                    ]
                    for c in range(n_dc):
                        for oh in range(n_oh):
                            nc.tensor.matmul(
                                out_pss[oh],
                                lhsT=sumT[:, h * D + c * P : h * D + c * P + P],
                                rhs=w_sb[:, c * O + oh * 512 : c * O + oh * 512 + 512],
                                start=(c == 0),
                                stop=(c == n_dc - 1),
                            )
                    for oh in range(n_oh):
                        nc.scalar.activation(
                            out_sb[:, h * O + oh * 512 : h * O + oh * 512 + 512],
                            out_pss[oh],
                            mybir.ActivationFunctionType.Relu,
                        )
                if i == nt - 1:
                    # split the last tile's store to shorten the tail
                    for h in range(H):
                        nc.scalar.dma_start(
                            out_h[bass.ts(i, P), h * O : (h + 1) * O],
                            out_sb[:, h * O : (h + 1) * O],
                        )
                else:
                    nc.scalar.dma_start(out_h[bass.ts(i, P), :], out_sb)

            # 3-stage software pipeline: load+reduce @ i, PE accumulate @
            # i-1, transpose+GEMM @ i-2
            for i in range(nt + 2):
                if i < nt:
                    stage_load(i)
                    stage_reduce(i)
                if 1 <= i < nt + 1:
                    stage_pe(i - 1)
                if i >= 2:
                    stage_gemm(i - 2)

    nc.compile()
    return nc


def shard_inputs(inputs: dict) -> list[dict]:
    n = inputs["self_vecs"].shape[0]
    per = n // N_CORES
    # pack [neigh ; self] as [N, G, D] f32; quantize with error feedback
    # along the group axis: fp8 groups first (carrying the rounding
    # residual forward), bf16 groups last (absorbing the residual at
    # bf16 precision)
    x = np.empty((n, G, D), dtype=np.float32)
    x[:, :K] = inputs["neigh_vecs"]
    x[:, K] = inputs["self_vecs"]
    pk8 = np.empty((n, G8, D), dtype=NP_F8)
    pkb = np.empty((n, K_BF, D), dtype=NP_BF)
    e = np.zeros((n, D), dtype=np.float32)
    for g in range(G8):
        t = x[:, g] + e
        pk8[:, g] = t.astype(NP_F8)
        e = t - pk8[:, g].astype(np.float32)
    for g in range(K_BF):
        t = x[:, G8 + g] + e
        pkb[:, g] = t.astype(NP_BF)
        e = t - pkb[:, g].astype(np.float32)
    # re-tile: node = tile*256 + h*128 + p -> row (tile*128+p), group-major
    # with the two halves of a group adjacent: [tile, p, g, h, 512]
    def retile(a, ng):
        return np.ascontiguousarray(
            a.reshape(n // (P * H), H, P, ng, D).transpose(0, 2, 3, 1, 4)
        ).reshape(n // H, ng * H * D)

    pk8 = retile(pk8, G8)
    pkb = retile(pkb, K_BF)
    # pre-rearrange W to [p, c, o] so the device load is contiguous rows
    w_bf = np.ascontiguousarray(
        inputs["W"].reshape(D // P, P, O).transpose(1, 0, 2).reshape(P, -1),
        dtype=NP_BF,
    )
    rows_per = per // H
    maps = []
    for c in range(N_CORES):
        sl = slice(c * rows_per, (c + 1) * rows_per)
        maps.append({"pk8": pk8[sl], "pkb": pkb[sl], "W": w_bf})
    return maps


def run_sharded(inputs: dict, trace: bool = False, **kwargs):
    from concourse.bass_utils import run_bass_kernel_spmd

    in_maps = shard_inputs(inputs)
    n_nodes = in_maps[0]["pk8"].shape[0] * H
    nc = build_nc(n_nodes)
    res = run_bass_kernel_spmd(
        nc, in_maps, core_ids=list(range(N_CORES)), trace=trace, **kwargs
    )
    # out rows: [tile*128+p, h*O:(h+1)*O] -> node tile*256+h*128+p
    outs = []
    for c in range(N_CORES):
        o = res.results[c]["out"]  # [rows_per, H*O]
        nt = o.shape[0] // P
        outs.append(
            o.reshape(nt, P, H, O).transpose(0, 2, 1, 3).reshape(-1, O)
        )
    out = np.concatenate(outs, axis=0).astype(np.float32)
    return out, res


def kernel(**inputs) -> np.ndarray:
    out, _ = run_sharded(inputs, trace=False)
    return out


# revision 9
# speedup vs baseline: 1.0168x; 1.0168x over previous
"""GCN aggregator kernel for Trainium2 (Bass/Tile), 8-core data-parallel.

Computes: out = relu(((sum_g x[:,g,:]) / (K+1)) @ W + b), x = [neigh;self]
Sharding: nodes (N) split evenly across 8 NeuronCores; W replicated. b is
zeros per the problem spec and is dropped on device.

The kernel is HBM-bandwidth bound (~320-360 GB/s/NC, the 8-core share of
the chip's HBM). The rel-err budget (2e-2) allows fp8: most of the
neighbor stream is cast to fp8_e3m4 (4 mantissa bits) on the host with
error-feedback rounding along the group axis (residual carried
group-to-group, absorbed by the final bf16 groups), which keeps
end-to-end max rel err at the bf16-baseline level (~4e-3) while cutting
HBM traffic ~45%.

fp8 costs compute: DVE runs 1-byte ops at 1x (no fp8 packing on TRN2),
so the 26-group reduction no longer fits on DVE alone in the shortened
DMA period. The reduction is split across engines per 128-node tile:
  - K_BF groups stay bf16 in the stream (DVE adds them at 2x; all DVE
    bf16 ops use flat contiguous slices - strided APs defeat 2x mode)
  - Q_POOL fp8 groups pair-add + self-merge on GPSIMD (idle otherwise);
    its single partial merges last in DVE's tree (no DVE stall)
  - P_PE fp8 groups are node-major matmul-accumulated (lhsT=I) into a
    PSUM f32 tile by the PE (warm under sustained load); DVE's merged
    partial is injected there as one more matmul
  - DVE pair-adds the rest fp8->bf16 + folds the partial tree, ordered
    by data arrival (fp8 slab merges first, late-landing bf16 slab
    after, Pool's partial last)
Then: ACT scaled-copy (1/26) PSUM->bf16, 4 PE transposes, ACT copy to
SBUF, PE GEMM vs bf16 W (f32 PSUM), ACT relu, bf16 store.

The emission is software-pipelined in 3 stages (load+DVE/Pool reduce @
tile i, PE accumulate @ i-1, transpose+GEMM+relu+store @ i-2) so the
in-order PE stream never blocks on the per-tile PE<->ACT ping-pong and
each PSUM accumulation group stays contiguous in the PE stream, and the
neighbor load is split into consumer-ordered slabs (pool | pe | dve |
bf16) so each engine's data lands just before it runs.

Host: fp8/bf16 packing + error feedback in numpy; group order arranged
so every engine operand is a flat contiguous slice; W pre-rearranged to
[p, c, o] so its load is contiguous; bf16 output upcast to f32.
"""

import os
import sys

import numpy as np
import ml_dtypes

for _p in ("/opt/trn_rl_repo", "/root/.axon_site/_ro/trn_rl_repo"):
    if os.path.isdir(_p) and _p not in sys.path:
        sys.path.insert(0, _p)

import concourse.bass as bass
import concourse.tile as tile
from concourse import bacc, mybir
from concourse.masks import make_identity

N, K, D, O = 16384, 25, 512, 1024
G = K + 1  # neigh groups + self
N_CORES = 8
P = 128  # nodes per tile (partition count)
INV = 1.0 / (K + 1)
FP = mybir.dt.float32
BF = mybir.dt.bfloat16
F8 = mybir.dt.float8e3
NP_BF = ml_dtypes.bfloat16
NP_F8 = ml_dtypes.float8_e3m4

# reduction split (groups): GPSIMD | PE | DVE fp8 | DVE bf16
Q_POOL = 6
P_PE = 10
K_BF = 4
R_DVE = G - Q_POOL - P_PE - K_BF  # 6
G8 = G - K_BF  # fp8 groups in the stream (22)
PEB = Q_POOL + P_PE  # end of PE slab (16)
# partial slots: 0-2 DVE fp8, 3-4 DVE bf16, 5 GPSIMD
NPART = R_DVE // 2 + K_BF // 2 + 1  # 6
assert R_DVE % 2 == 0 and Q_POOL % 2 == 0 and K_BF % 2 == 0


def build_nc(n_nodes: int, neigh_bufs: int = 6) -> bass.Bass:
    """Build the per-core Bass program for a shard of `n_nodes` nodes."""
    assert n_nodes % P == 0
    nt = n_nodes // P

    nc = bacc.Bacc("TRN2", target_bir_lowering=False, debug=False)
    pk8_h = nc.dram_tensor("pk8", [n_nodes, G8 * D], F8, kind="ExternalInput")
    pkb_h = nc.dram_tensor("pkb", [n_nodes, K_BF * D], BF, kind="ExternalInput")
    # W pre-rearranged on host to [p, c, o] (row p = partition line of every
    # d-chunk's rhs) so the device load is contiguous 8KB rows
    w_h = nc.dram_tensor("W", [P, (D // P) * O], BF, kind="ExternalInput")
    out_h = nc.dram_tensor("out", [n_nodes, O], BF, kind="ExternalOutput")

    n_dc = D // P  # d-chunks for transposes / GEMM contraction
    n_oh = O // 512

    def g8(t, a, b):  # flat slice of D-wide group units [a, b)
        return t[:, a * D : b * D]

    with tile.TileContext(nc) as tc:
        with (
            tc.tile_pool(name="const", bufs=1) as const_pool,
            tc.tile_pool(name="neigh", bufs=neigh_bufs) as neigh_pool,
            tc.tile_pool(name="parts", bufs=3) as parts_pool,
            tc.tile_pool(name="pp", bufs=3) as pp_pool,
            tc.tile_pool(name="small", bufs=3) as small_pool,
            tc.tile_pool(name="outp", bufs=3) as out_pool,
            tc.tile_pool(name="ps_a", bufs=2, space="PSUM") as ps_a_pool,
            tc.tile_pool(name="ps_t", bufs=2, space="PSUM") as ps_t_pool,
            tc.tile_pool(name="ps_o", bufs=2, space="PSUM") as ps_o_pool,
        ):
            w_sb = const_pool.tile([P, n_dc * O], BF)
            ident = const_pool.tile([P, P], BF)
            make_identity(nc, ident)
            # W rides the scalar hwdge queue, parallel to the neigh stream
            nc.scalar.dma_start(w_sb, w_h[:, :])

            nh8s, parts_l, psA_l, means_l = {}, {}, {}, {}

            def stage_load(i):
                nh8 = neigh_pool.tile([P, G8 * D], F8, tag="nh8", name="nh8")
                rows = bass.ts(i, P)
                # consumer-ordered slabs: pool | pe | dve-fp8 | dve-bf16
                nc.sync.dma_start(g8(nh8, 0, Q_POOL), pk8_h[rows, : Q_POOL * D])
                nc.sync.dma_start(
                    g8(nh8, Q_POOL, PEB), pk8_h[rows, Q_POOL * D : PEB * D]
                )
                nc.sync.dma_start(g8(nh8, PEB, G8), pk8_h[rows, PEB * D :])
                nhb = neigh_pool.tile([P, K_BF * D], BF, tag="nhb", name="nhb")
                nc.sync.dma_start(nhb, pkb_h[rows, :])
                nh8s[i] = (nh8, nhb)

            def stage_reduce(i):
                nh8, nhb = nh8s[i]
                parts = parts_pool.tile([P, NPART * D], BF, tag="pt", name="pt")
                parts_l[i] = parts
                # GPSIMD: groups [0,6) pair-added then self-merged -> slot 5
                pp = pp_pool.tile([P, 3 * D], BF, tag="pp", name="pp")
                nc.gpsimd.tensor_add(pp, g8(nh8, 0, 3), g8(nh8, 3, 6))
                nc.gpsimd.tensor_add(g8(pp, 0, 1), g8(pp, 0, 1), g8(pp, 1, 2))
                nc.gpsimd.tensor_add(g8(parts, 5, 6), g8(pp, 0, 1), g8(pp, 2, 3))
                # DVE, ordered by data arrival: fp8 pairs + their merges
                # first, late-landing bf16 slab next, Pool's partial last
                nc.vector.tensor_add(
                    g8(parts, 0, 3), g8(nh8, PEB, PEB + 3), g8(nh8, PEB + 3, G8)
                )
                nc.vector.tensor_add(g8(parts, 0, 1), g8(parts, 0, 1), g8(parts, 1, 2))
                nc.vector.tensor_add(g8(parts, 0, 1), g8(parts, 0, 1), g8(parts, 2, 3))
                nc.vector.tensor_add(
                    g8(parts, 3, 5), nhb[:, 0 : 2 * D], nhb[:, 2 * D : 4 * D]
                )
                nc.vector.tensor_add(g8(parts, 3, 4), g8(parts, 3, 4), g8(parts, 4, 5))
                nc.vector.tensor_add(g8(parts, 0, 1), g8(parts, 0, 1), g8(parts, 3, 4))
                nc.vector.tensor_add(g8(parts, 0, 1), g8(parts, 0, 1), g8(parts, 5, 6))

            def stage_pe(i):
                # PE: node-major accumulate raw fp8 groups + DVE's merged
                # partial (lhsT=I) in one contiguous PSUM accumulation group
                # (runs one tile behind the DVE frontier, so no PE stall)
                nh8, _ = nh8s[i]
                psA = ps_a_pool.tile([P, D], FP, tag="psA", name="psA")
                psA_l[i] = psA
                for j in range(P_PE):
                    nc.tensor.matmul(
                        psA,
                        lhsT=ident,
                        rhs=g8(nh8, Q_POOL + j, Q_POOL + j + 1),
                        start=(j == 0),
                        stop=False,
                    )
                nc.tensor.matmul(
                    psA, lhsT=ident, rhs=g8(parts_l[i], 0, 1),
                    start=False, stop=True,
                )
                means = small_pool.tile([P, D], BF, tag="mn", name="mn")
                means_l[i] = means
                nc.scalar.activation(
                    means, psA, mybir.ActivationFunctionType.Copy, scale=INV
                )

            def stage_gemm(i):
                means = means_l.pop(i)
                sumT = small_pool.tile([P, D], BF, tag="tsb", name="tsb")
                tps = ps_t_pool.tile([P, D], BF, tag="tps", name="tps")
                for c in range(n_dc):
                    nc.tensor.transpose(
                        tps[:, bass.ts(c, P)], means[:, bass.ts(c, P)], ident
                    )
                nc.scalar.activation(sumT, tps, mybir.ActivationFunctionType.Copy)
                out_pss = [
                    ps_o_pool.tile([P, 512], FP, tag=f"ops{oh}", name=f"ops{oh}")
                    for oh in range(n_oh)
                ]
                for c in range(n_dc):
                    for oh in range(n_oh):
                        nc.tensor.matmul(
                            out_pss[oh],
                            lhsT=sumT[:, bass.ts(c, P)],
                            rhs=w_sb[:, c * O + oh * 512 : c * O + oh * 512 + 512],
                            start=(c == 0),
                            stop=(c == n_dc - 1),
                        )
                out_sb = out_pool.tile([P, O], BF)
                for oh in range(n_oh):
                    nc.scalar.activation(
                        out_sb[:, bass.ts(oh, 512)],
                        out_pss[oh],
                        mybir.ActivationFunctionType.Relu,
                    )
                    if i == nt - 1:
                        nc.scalar.dma_start(
                            out_h[bass.ts(i, P), bass.ts(oh, 512)],
                            out_sb[:, bass.ts(oh, 512)],
                        )
                if i != nt - 1:
                    nc.scalar.dma_start(out_h[bass.ts(i, P), :], out_sb)

            # 3-stage software pipeline: load+reduce @ i, PE accumulate @
            # i-1, transpose+GEMM @ i-2
            for i in range(nt + 2):
                if i < nt:
                    stage_load(i)
                    stage_reduce(i)
                if 1 <= i < nt + 1:
                    stage_pe(i - 1)
                if i >= 2:
                    stage_gemm(i - 2)

    nc.compile()
    return nc


def shard_inputs(inputs: dict) -> list[dict]:
    n = inputs["self_vecs"].shape[0]
    per = n // N_CORES
    # pack [neigh ; self] as [N, G, D] f32; quantize with error feedback
    # along the group axis: fp8 groups first (carrying the rounding
    # residual forward), bf16 groups last (absorbing the residual at
    # bf16 precision)
    x = np.empty((n, G, D), dtype=np.float32)
    x[:, :K] = inputs["neigh_vecs"]
    x[:, K] = inputs["self_vecs"]
    pk8 = np.empty((n, G8, D), dtype=NP_F8)
    pkb = np.empty((n, K_BF, D), dtype=NP_BF)
    e = np.zeros((n, D), dtype=np.float32)
    for g in range(G8):
        t = x[:, g] + e
        pk8[:, g] = t.astype(NP_F8)
        e = t - pk8[:, g].astype(np.float32)
    for g in range(K_BF):
        t = x[:, G8 + g] + e
        pkb[:, g] = t.astype(NP_BF)
        e = t - pkb[:, g].astype(np.float32)
    # pre-rearrange W to [p, c, o] so the device load is contiguous rows
    w_bf = np.ascontiguousarray(
        inputs["W"].reshape(D // P, P, O).transpose(1, 0, 2).reshape(P, -1),
        dtype=NP_BF,
    )
    pk8 = pk8.reshape(n, G8 * D)
    pkb = pkb.reshape(n, K_BF * D)
    maps = []
    for c in range(N_CORES):
        sl = slice(c * per, (c + 1) * per)
        maps.append({"pk8": pk8[sl], "pkb": pkb[sl], "W": w_bf})
    return maps


def run_sharded(inputs: dict, trace: bool = False, **kwargs):
    from concourse.bass_utils import run_bass_kernel_spmd

    in_maps = shard_inputs(inputs)
    n_nodes = in_maps[0]["pk8"].shape[0]
    nc = build_nc(n_nodes)
    res = run_bass_kernel_spmd(
        nc, in_maps, core_ids=list(range(N_CORES)), trace=trace, **kwargs
    )
    out = np.concatenate(
        [res.results[c]["out"] for c in range(N_CORES)], axis=0
    ).astype(np.float32)
    return out, res


def kernel(**inputs) -> np.ndarray:
    out, _ = run_sharded(inputs, trace=False)
    return out


# revision 12
# speedup vs baseline: 1.1572x; 1.1381x over previous
"""GCN aggregator kernel for Trainium2 (Bass/Tile), 8-core data-parallel.

Computes: out = relu(((sum_g x[:,g,:]) / (K+1)) @ W + b), x = [neigh;self]
Sharding: nodes (N) split evenly across 8 NeuronCores; W replicated. b is
zeros per the problem spec and is dropped on device.

The kernel is HBM-bandwidth bound (~320-360 GB/s/NC, the 8-core share of
the chip's HBM). The rel-err budget (2e-2) allows fp8: most of the
neighbor stream is cast to fp8_e3m4 (4 mantissa bits) on the host with
error-feedback rounding along the group axis (residual carried
group-to-group, absorbed by the final bf16 groups), which keeps
end-to-end max rel err at the bf16-baseline level (~4e-3) while cutting
HBM traffic ~45%.

fp8 costs compute: DVE runs 1-byte ops at 1x (no fp8 packing on TRN2),
so the 26-group reduction no longer fits on DVE alone in the shortened
DMA period. The reduction is split across engines per 128-node tile:
  - K_BF groups stay bf16 in the stream (DVE adds them at 2x; all DVE
    bf16 ops use flat contiguous slices - strided APs defeat 2x mode)
  - Q_POOL fp8 groups pair-add on GPSIMD (idle otherwise) in one flat
    batched op; its 3 partials merge in DVE's tree
  - P_PE fp8 groups are node-major matmul-accumulated (lhsT=I) into a
    PSUM f32 tile by the PE (warm under sustained load); DVE's merged
    partial is injected there as one more matmul
  - DVE pair-adds the rest fp8->bf16 + folds the partial tree
Then: ACT scaled-copy (1/26) PSUM->bf16, 4 PE transposes, ACT copy to
SBUF, PE GEMM vs bf16 W (f32 PSUM), ACT relu, bf16 store.

The emission is software-pipelined in 3 stages (load+DVE/Pool reduce @
tile i, PE accumulate @ i-1, transpose+GEMM+relu+store @ i-2) so the
in-order PE stream never blocks on the per-tile PE<->ACT ping-pong and
each PSUM accumulation group stays contiguous in the PE stream, and the
neighbor load is split into consumer-ordered slabs (pool | pe | dve |
bf16) so each engine's data lands just before it runs.

Host: fp8/bf16 packing + error feedback in numpy; group order arranged
so every engine operand is a flat contiguous slice; W pre-rearranged to
[p, c, o] so its load is contiguous; bf16 output upcast to f32.
"""

import os
import sys

import numpy as np
import ml_dtypes

for _p in ("/opt/trn_rl_repo", "/root/.axon_site/_ro/trn_rl_repo"):
    if os.path.isdir(_p) and _p not in sys.path:
        sys.path.insert(0, _p)

import concourse.bass as bass
import concourse.tile as tile
from concourse import bacc, mybir
from concourse.masks import make_identity

N, K, D, O = 16384, 25, 512, 1024
G = K + 1  # neigh groups + self
N_CORES = 8
P = 128  # nodes per tile (partition count)
INV = 1.0 / (K + 1)
FP = mybir.dt.float32
BF = mybir.dt.bfloat16
F8 = mybir.dt.float8e3
NP_BF = ml_dtypes.bfloat16
NP_F8 = ml_dtypes.float8_e3m4

# reduction split (groups): GPSIMD | PE | DVE fp8 | DVE bf16
Q_POOL = 6
P_PE = 10
K_BF = 4
R_DVE = G - Q_POOL - P_PE - K_BF  # 6
G8 = G - K_BF  # fp8 groups in the stream (22)
PEB = Q_POOL + P_PE  # end of PE slab (16)
# partial slots: 0-2 DVE fp8, 3-5 GPSIMD, 6-7 DVE bf16
NPART = R_DVE // 2 + Q_POOL // 2 + K_BF // 2  # 8
assert R_DVE % 2 == 0 and Q_POOL % 2 == 0 and K_BF % 2 == 0


def build_nc(n_nodes: int, neigh_bufs: int = 6) -> bass.Bass:
    """Build the per-core Bass program for a shard of `n_nodes` nodes."""
    assert n_nodes % P == 0
    nt = n_nodes // P

    nc = bacc.Bacc("TRN2", target_bir_lowering=False, debug=False)
    pk8_h = nc.dram_tensor("pk8", [n_nodes, G8 * D], F8, kind="ExternalInput")
    pkb_h = nc.dram_tensor("pkb", [n_nodes, K_BF * D], BF, kind="ExternalInput")
    # W pre-rearranged on host to [p, c, o] (row p = partition line of every
    # d-chunk's rhs) so the device load is contiguous 8KB rows
    w_h = nc.dram_tensor("W", [P, (D // P) * O], BF, kind="ExternalInput")
    out_h = nc.dram_tensor("out", [n_nodes, O], BF, kind="ExternalOutput")

    n_dc = D // P  # d-chunks for transposes / GEMM contraction
    n_oh = O // 512

    def g8(t, a, b):  # flat slice of D-wide group units [a, b)
        return t[:, a * D : b * D]

    with tile.TileContext(nc) as tc:
        with (
            tc.tile_pool(name="const", bufs=1) as const_pool,
            tc.tile_pool(name="neigh", bufs=neigh_bufs) as neigh_pool,
            tc.tile_pool(name="parts", bufs=3) as parts_pool,
            tc.tile_pool(name="small", bufs=3) as small_pool,
            tc.tile_pool(name="outp", bufs=3) as out_pool,
            tc.tile_pool(name="ps_a", bufs=2, space="PSUM") as ps_a_pool,
            tc.tile_pool(name="ps_t", bufs=2, space="PSUM") as ps_t_pool,
            tc.tile_pool(name="ps_o", bufs=2, space="PSUM") as ps_o_pool,
        ):
            w_sb = const_pool.tile([P, n_dc * O], BF)
            ident = const_pool.tile([P, P], BF)
            make_identity(nc, ident)
            # W rides the scalar hwdge queue, parallel to the neigh stream
            nc.scalar.dma_start(w_sb, w_h[:, :])

            nh8s, parts_l, psA_l, means_l = {}, {}, {}, {}

            def stage_load(i):
                nh8 = neigh_pool.tile([P, G8 * D], F8, tag="nh8", name="nh8")
                rows = bass.ts(i, P)
                # consumer-ordered slabs: pool | pe | dve-fp8 | dve-bf16
                nc.sync.dma_start(g8(nh8, 0, Q_POOL), pk8_h[rows, : Q_POOL * D])
                nc.sync.dma_start(
                    g8(nh8, Q_POOL, PEB), pk8_h[rows, Q_POOL * D : PEB * D]
                )
                nc.sync.dma_start(g8(nh8, PEB, G8), pk8_h[rows, PEB * D :])
                nhb = neigh_pool.tile([P, K_BF * D], BF, tag="nhb", name="nhb")
                nc.sync.dma_start(nhb, pkb_h[rows, :])
                nh8s[i] = (nh8, nhb)

            def stage_reduce(i):
                nh8, nhb = nh8s[i]
                parts = parts_pool.tile([P, NPART * D], BF, tag="pt", name="pt")
                parts_l[i] = parts
                # GPSIMD: groups [0,6) -> slots 3-5 (one flat batched op)
                nc.gpsimd.tensor_add(
                    g8(parts, 3, 6), g8(nh8, 0, 3), g8(nh8, 3, 6)
                )
                # DVE: fp8 pairs -> slots 0-2, bf16 pairs -> slots 6-7
                nc.vector.tensor_add(
                    g8(parts, 0, 3), g8(nh8, PEB, PEB + 3), g8(nh8, PEB + 3, G8)
                )
                nc.vector.tensor_add(
                    g8(parts, 6, 8), nhb[:, 0 : 2 * D], nhb[:, 2 * D : 4 * D]
                )
                # tree: {0..3}+{4..7} -> {0,1}+{2,3} -> {0}+{1}
                nc.vector.tensor_add(g8(parts, 0, 4), g8(parts, 0, 4), g8(parts, 4, 8))
                nc.vector.tensor_add(g8(parts, 0, 2), g8(parts, 0, 2), g8(parts, 2, 4))
                nc.vector.tensor_add(g8(parts, 0, 1), g8(parts, 0, 1), g8(parts, 1, 2))

            def stage_pe(i):
                # PE: node-major accumulate raw fp8 groups + DVE's merged
                # partial (lhsT=I) in one contiguous PSUM accumulation group
                # (runs one tile behind the DVE frontier, so no PE stall)
                nh8, _ = nh8s[i]
                psA = ps_a_pool.tile([P, D], FP, tag="psA", name="psA")
                psA_l[i] = psA
                for j in range(P_PE):
                    nc.tensor.matmul(
                        psA,
                        lhsT=ident,
                        rhs=g8(nh8, Q_POOL + j, Q_POOL + j + 1),
                        start=(j == 0),
                        stop=False,
                    )
                nc.tensor.matmul(
                    psA, lhsT=ident, rhs=g8(parts_l[i], 0, 1),
                    start=False, stop=True,
                )
                means = small_pool.tile([P, D], BF, tag="mn", name="mn")
                means_l[i] = means
                nc.scalar.activation(
                    means, psA, mybir.ActivationFunctionType.Copy, scale=INV
                )

            def stage_gemm(i):
                means = means_l.pop(i)
                sumT = small_pool.tile([P, D], BF, tag="tsb", name="tsb")
                tps = ps_t_pool.tile([P, D], BF, tag="tps", name="tps")
                for c in range(n_dc):
                    nc.tensor.transpose(
                        tps[:, bass.ts(c, P)], means[:, bass.ts(c, P)], ident
                    )
                nc.scalar.activation(sumT, tps, mybir.ActivationFunctionType.Copy)
                out_pss = [
                    ps_o_pool.tile([P, 512], FP, tag=f"ops{oh}", name=f"ops{oh}")
                    for oh in range(n_oh)
                ]
                for c in range(n_dc):
                    for oh in range(n_oh):
                        nc.tensor.matmul(
                            out_pss[oh],
                            lhsT=sumT[:, bass.ts(c, P)],
                            rhs=w_sb[:, c * O + oh * 512 : c * O + oh * 512 + 512],
                            start=(c == 0),
                            stop=(c == n_dc - 1),
                        )
                out_sb = out_pool.tile([P, O], BF)
                for oh in range(n_oh):
                    nc.scalar.activation(
                        out_sb[:, bass.ts(oh, 512)],
                        out_pss[oh],
                        mybir.ActivationFunctionType.Relu,
                    )
                    if i == nt - 1:
                        nc.scalar.dma_start(
                            out_h[bass.ts(i, P), bass.ts(oh, 512)],
                            out_sb[:, bass.ts(oh, 512)],
                        )
                if i != nt - 1:
                    nc.scalar.dma_start(out_h[bass.ts(i, P), :], out_sb)

            # 3-stage software pipeline: load+reduce @ i, PE accumulate @
            # i-1, transpose+GEMM @ i-2
            for i in range(nt + 2):
                if i < nt:
                    stage_load(i)
                    stage_reduce(i)
                if 1 <= i < nt + 1:
                    stage_pe(i - 1)
                if i >= 2:
                    stage_gemm(i - 2)

    nc.compile()
    return nc


def shard_inputs(inputs: dict) -> list[dict]:
    n = inputs["self_vecs"].shape[0]
    per = n // N_CORES
    # pack [neigh ; self] as [N, G, D] f32; quantize with error feedback
    # along the group axis: fp8 groups first (carrying the rounding
    # residual forward), bf16 groups last (absorbing the residual at
    # bf16 precision)
    x = np.empty((n, G, D), dtype=np.float32)
    x[:, :K] = inputs["neigh_vecs"]
    x[:, K] = inputs["self_vecs"]
    pk8 = np.empty((n, G8, D), dtype=NP_F8)
    pkb = np.empty((n, K_BF, D), dtype=NP_BF)
    e = np.zeros((n, D), dtype=np.float32)
    for g in range(G8):
        t = x[:, g] + e
        pk8[:, g] = t.astype(NP_F8)
        e = t - pk8[:, g].astype(np.float32)
    for g in range(K_BF):
        t = x[:, G8 + g] + e
        pkb[:, g] = t.astype(NP_BF)
        e = t - pkb[:, g].astype(np.float32)
    # pre-rearrange W to [p, c, o] so the device load is contiguous rows
    w_bf = np.ascontiguousarray(
        inputs["W"].reshape(D // P, P, O).transpose(1, 0, 2).reshape(P, -1),
        dtype=NP_BF,
    )
    pk8 = pk8.reshape(n, G8 * D)
    pkb = pkb.reshape(n, K_BF * D)
    maps = []
    for c in range(N_CORES):
        sl = slice(c * per, (c + 1) * per)
        maps.append({"pk8": pk8[sl], "pkb": pkb[sl], "W": w_bf})
    return maps


def run_sharded(inputs: dict, trace: bool = False, **kwargs):
    from concourse.bass_utils import run_bass_kernel_spmd

    in_maps = shard_inputs(inputs)
    n_nodes = in_maps[0]["pk8"].shape[0]
    nc = build_nc(n_nodes)
    res = run_bass_kernel_spmd(
        nc, in_maps, core_ids=list(range(N_CORES)), trace=trace, **kwargs
    )
    out = np.concatenate(
        [res.results[c]["out"] for c in range(N_CORES)], axis=0
    ).astype(np.float32)
    return out, res


def kernel(**inputs) -> np.ndarray:
    out, _ = run_sharded(inputs, trace=False)
    return out


# revision 15
# speedup vs baseline: 1.1604x; 1.0028x over previous
"""GCN aggregator kernel for Trainium2 (Bass/Tile), 8-core data-parallel.

Computes: out = relu(((sum_g x[:,g,:]) / (K+1)) @ W + b), x = [neigh;self]
Sharding: nodes (N) split evenly across 8 NeuronCores; W replicated. b is
zeros per the problem spec and is dropped on device.

The kernel is HBM-bandwidth bound (~320-360 GB/s/NC, the 8-core share of
the chip's HBM). The rel-err budget (2e-2) allows fp8: most of the
neighbor stream is cast to fp8_e3m4 (4 mantissa bits) on the host with
error-feedback rounding along the group axis (residual carried
group-to-group, absorbed by the final bf16 groups), which keeps
end-to-end max rel err at the bf16-baseline level (~4e-3) while cutting
HBM traffic ~45%.

fp8 costs compute: DVE runs 1-byte ops at 1x (no fp8 packing on TRN2),
so the 26-group reduction no longer fits on DVE alone in the shortened
DMA period. The reduction is split across engines per 128-node tile:
  - K_BF groups stay bf16 in the stream (DVE adds them at 2x; all DVE
    bf16 ops use flat contiguous slices - strided APs defeat 2x mode)
  - Q_POOL fp8 groups pair-add on GPSIMD (idle otherwise) in one flat
    batched op; its 3 partials merge in DVE's tree
  - P_PE fp8 groups are node-major matmul-accumulated (lhsT=I) into a
    PSUM f32 tile by the PE (warm under sustained load); DVE's merged
    partial is injected there as one more matmul
  - DVE pair-adds the rest fp8->bf16 + folds the partial tree
Then: ACT scaled-copy (1/26) PSUM->bf16, 4 PE transposes, ACT copy to
SBUF, PE GEMM vs bf16 W (f32 PSUM), ACT relu, bf16 store.

The emission is software-pipelined in 3 stages (load+DVE/Pool reduce @
tile i, PE accumulate @ i-1, transpose+GEMM+relu+store @ i-2) so the
in-order PE stream never blocks on the per-tile PE<->ACT ping-pong and
each PSUM accumulation group stays contiguous in the PE stream, and the
neighbor load is split into consumer-ordered slabs (pool | pe | dve |
bf16) so each engine's data lands just before it runs.

Host: fp8/bf16 packing + error feedback in numpy; group order arranged
so every engine operand is a flat contiguous slice; W pre-rearranged to
[p, c, o] so its load is contiguous; bf16 output upcast to f32.
"""

import os
import sys

import numpy as np
import ml_dtypes

for _p in ("/opt/trn_rl_repo", "/root/.axon_site/_ro/trn_rl_repo"):
    if os.path.isdir(_p) and _p not in sys.path:
        sys.path.insert(0, _p)

import concourse.bass as bass
import concourse.tile as tile
from concourse import bacc, mybir
from concourse.masks import make_identity

N, K, D, O = 16384, 25, 512, 1024
G = K + 1  # neigh groups + self
N_CORES = 8
P = 128  # nodes per tile (partition count)
INV = 1.0 / (K + 1)
FP = mybir.dt.float32
BF = mybir.dt.bfloat16
F8 = mybir.dt.float8e3
NP_BF = ml_dtypes.bfloat16
NP_F8 = ml_dtypes.float8_e3m4

# reduction split (groups): GPSIMD | PE | DVE fp8 | DVE bf16
Q_POOL = 6
P_PE = 10
K_BF = 4
R_DVE = G - Q_POOL - P_PE - K_BF  # 6
G8 = G - K_BF  # fp8 groups in the stream (22)
PEB = Q_POOL + P_PE  # end of PE slab (16)
# partial slots: 0-2 DVE fp8, 3-5 GPSIMD, 6-7 DVE bf16
NPART = R_DVE // 2 + Q_POOL // 2 + K_BF // 2  # 8
assert R_DVE % 2 == 0 and Q_POOL % 2 == 0 and K_BF % 2 == 0


def build_nc(n_nodes: int, neigh_bufs: int = 6) -> bass.Bass:
    """Build the per-core Bass program for a shard of `n_nodes` nodes."""
    assert n_nodes % P == 0
    nt = n_nodes // P

    nc = bacc.Bacc("TRN2", target_bir_lowering=False, debug=False)
    pk8_h = nc.dram_tensor("pk8", [n_nodes, G8 * D], F8, kind="ExternalInput")
    pkb_h = nc.dram_tensor("pkb", [n_nodes, K_BF * D], BF, kind="ExternalInput")
    # W pre-rearranged on host to [p, c, o] (row p = partition line of every
    # d-chunk's rhs) so the device load is contiguous 8KB rows
    w_h = nc.dram_tensor("W", [P, (D // P) * O], BF, kind="ExternalInput")
    out_h = nc.dram_tensor("out", [n_nodes, O], BF, kind="ExternalOutput")

    n_dc = D // P  # d-chunks for transposes / GEMM contraction
    n_oh = O // 512

    def g8(t, a, b):  # flat slice of D-wide group units [a, b)
        return t[:, a * D : b * D]

    with tile.TileContext(nc) as tc:
        with (
            tc.tile_pool(name="const", bufs=1) as const_pool,
            tc.tile_pool(name="neigh", bufs=neigh_bufs) as neigh_pool,
            tc.tile_pool(name="parts", bufs=3) as parts_pool,
            tc.tile_pool(name="small", bufs=3) as small_pool,
            tc.tile_pool(name="outp", bufs=3) as out_pool,
            tc.tile_pool(name="ps_a", bufs=2, space="PSUM") as ps_a_pool,
            tc.tile_pool(name="ps_t", bufs=2, space="PSUM") as ps_t_pool,
            tc.tile_pool(name="ps_o", bufs=2, space="PSUM") as ps_o_pool,
        ):
            w_sb = const_pool.tile([P, n_dc * O], BF)
            ident = const_pool.tile([P, P], BF)
            make_identity(nc, ident)
            # W rides the scalar hwdge queue, parallel to the neigh stream
            nc.scalar.dma_start(w_sb, w_h[:, :])

            nh8s, parts_l, psA_l, means_l = {}, {}, {}, {}

            def stage_load(i):
                nh8 = neigh_pool.tile([P, G8 * D], F8, tag="nh8", name="nh8")
                rows = bass.ts(i, P)
                # loads split across both hwdge queues for DMA parallelism:
                # scalar gets pool slab + bf16 slab, sync gets pe + dve slabs
                nc.scalar.dma_start(g8(nh8, 0, Q_POOL), pk8_h[rows, : Q_POOL * D])
                nc.sync.dma_start(
                    g8(nh8, Q_POOL, PEB), pk8_h[rows, Q_POOL * D : PEB * D]
                )
                nc.sync.dma_start(g8(nh8, PEB, G8), pk8_h[rows, PEB * D :])
                nhb = neigh_pool.tile([P, K_BF * D], BF, tag="nhb", name="nhb")
                nc.scalar.dma_start(nhb, pkb_h[rows, :])
                nh8s[i] = (nh8, nhb)

            def stage_reduce(i):
                nh8, nhb = nh8s[i]
                parts = parts_pool.tile([P, NPART * D], BF, tag="pt", name="pt")
                parts_l[i] = parts
                # GPSIMD: groups [0,6) -> slots 5-7 (one flat batched op);
                # the PE absorbs these partials directly, so the DVE tree
                # never depends on GPSIMD
                nc.gpsimd.tensor_add(
                    g8(parts, 5, 8), g8(nh8, 0, 3), g8(nh8, 3, 6)
                )
                # DVE: fp8 pairs -> slots 0-2, bf16 pairs -> slots 3-4
                nc.vector.tensor_add(
                    g8(parts, 0, 3), g8(nh8, PEB, PEB + 3), g8(nh8, PEB + 3, G8)
                )
                nc.vector.tensor_add(
                    g8(parts, 3, 5), nhb[:, 0 : 2 * D], nhb[:, 2 * D : 4 * D]
                )
                # tree over DVE's own partials only: {0,1}+{2,3} -> {0}+{1}
                # -> {0}+{4}
                nc.vector.tensor_add(g8(parts, 0, 2), g8(parts, 0, 2), g8(parts, 2, 4))
                nc.vector.tensor_add(g8(parts, 0, 1), g8(parts, 0, 1), g8(parts, 1, 2))
                nc.vector.tensor_add(g8(parts, 0, 1), g8(parts, 0, 1), g8(parts, 4, 5))

            def stage_pe(i):
                # PE: node-major accumulate raw fp8 groups + DVE's merged
                # partial (lhsT=I) in one contiguous PSUM accumulation group
                # (runs one tile behind the DVE frontier, so no PE stall)
                nh8, _ = nh8s[i]
                psA = ps_a_pool.tile([P, D], FP, tag="psA", name="psA")
                psA_l[i] = psA
                for j in range(P_PE):
                    nc.tensor.matmul(
                        psA,
                        lhsT=ident,
                        rhs=g8(nh8, Q_POOL + j, Q_POOL + j + 1),
                        start=(j == 0),
                        stop=False,
                    )
                # inject DVE's merged partial + GPSIMD's three partials
                for s in (0, 5, 6, 7):
                    nc.tensor.matmul(
                        psA, lhsT=ident, rhs=g8(parts_l[i], s, s + 1),
                        start=False, stop=(s == 7),
                    )
                means = small_pool.tile([P, D], BF, tag="mn", name="mn")
                means_l[i] = means
                nc.scalar.activation(
                    means, psA, mybir.ActivationFunctionType.Copy, scale=INV
                )

            def stage_gemm(i):
                means = means_l.pop(i)
                sumT = small_pool.tile([P, D], BF, tag="tsb", name="tsb")
                tps = ps_t_pool.tile([P, D], BF, tag="tps", name="tps")
                for c in range(n_dc):
                    nc.tensor.transpose(
                        tps[:, bass.ts(c, P)], means[:, bass.ts(c, P)], ident
                    )
                nc.scalar.activation(sumT, tps, mybir.ActivationFunctionType.Copy)
                out_pss = [
                    ps_o_pool.tile([P, 512], FP, tag=f"ops{oh}", name=f"ops{oh}")
                    for oh in range(n_oh)
                ]
                for c in range(n_dc):
                    for oh in range(n_oh):
                        nc.tensor.matmul(
                            out_pss[oh],
                            lhsT=sumT[:, bass.ts(c, P)],
                            rhs=w_sb[:, c * O + oh * 512 : c * O + oh * 512 + 512],
                            start=(c == 0),
                            stop=(c == n_dc - 1),
                        )
                out_sb = out_pool.tile([P, O], BF)
                for oh in range(n_oh):
                    nc.scalar.activation(
                        out_sb[:, bass.ts(oh, 512)],
                        out_pss[oh],
                        mybir.ActivationFunctionType.Relu,
                    )
                    if i == nt - 1:
                        nc.scalar.dma_start(
                            out_h[bass.ts(i, P), bass.ts(oh, 512)],
                            out_sb[:, bass.ts(oh, 512)],
                        )
                if i != nt - 1:
                    nc.scalar.dma_start(out_h[bass.ts(i, P), :], out_sb)

            # 3-stage software pipeline: load+reduce @ i, PE accumulate @
            # i-1, transpose+GEMM @ i-2
            for i in range(nt + 2):
                if i < nt:
                    stage_load(i)
                    stage_reduce(i)
                if 1 <= i < nt + 1:
                    stage_pe(i - 1)
                if i >= 2:
                    stage_gemm(i - 2)

    nc.compile()
    return nc


def shard_inputs(inputs: dict) -> list[dict]:
    n = inputs["self_vecs"].shape[0]
    per = n // N_CORES
    # pack [neigh ; self] as [N, G, D] f32; quantize with error feedback
    # along the group axis: fp8 groups first (carrying the rounding
    # residual forward), bf16 groups last (absorbing the residual at
    # bf16 precision)
    x = np.empty((n, G, D), dtype=np.float32)
    x[:, :K] = inputs["neigh_vecs"]
    x[:, K] = inputs["self_vecs"]
    pk8 = np.empty((n, G8, D), dtype=NP_F8)
    pkb = np.empty((n, K_BF, D), dtype=NP_BF)
    e = np.zeros((n, D), dtype=np.float32)
    for g in range(G8):
        t = x[:, g] + e
        pk8[:, g] = t.astype(NP_F8)
        e = t - pk8[:, g].astype(np.float32)
    for g in range(K_BF):
        t = x[:, G8 + g] + e
        pkb[:, g] = t.astype(NP_BF)
        e = t - pkb[:, g].astype(np.float32)
    # pre-rearrange W to [p, c, o] so the device load is contiguous rows
    w_bf = np.ascontiguousarray(
        inputs["W"].reshape(D // P, P, O).transpose(1, 0, 2).reshape(P, -1),
        dtype=NP_BF,
    )
    pk8 = pk8.reshape(n, G8 * D)
    pkb = pkb.reshape(n, K_BF * D)
    maps = []
    for c in range(N_CORES):
        sl = slice(c * per, (c + 1) * per)
        maps.append({"pk8": pk8[sl], "pkb": pkb[sl], "W": w_bf})
    return maps


def run_sharded(inputs: dict, trace: bool = False, **kwargs):
    from concourse.bass_utils import run_bass_kernel_spmd

    in_maps = shard_inputs(inputs)
    n_nodes = in_maps[0]["pk8"].shape[0]
    nc = build_nc(n_nodes)
    res = run_bass_kernel_spmd(
        nc, in_maps, core_ids=list(range(N_CORES)), trace=trace, **kwargs
    )
    out = np.concatenate(
        [res.results[c]["out"] for c in range(N_CORES)], axis=0
    ).astype(np.float32)
    return out, res


def kernel(**inputs) -> np.ndarray:
    out, _ = run_sharded(inputs, trace=False)
    return out


# revision 17
# speedup vs baseline: 1.2702x; 1.0946x over previous
"""GCN aggregator kernel for Trainium2 (Bass/Tile), 8-core data-parallel.

Computes: out = relu(((sum_g x[:,g,:]) / (K+1)) @ W + b), x = [neigh;self]
Sharding: nodes (N) split evenly across 8 NeuronCores; W replicated. b is
zeros per the problem spec and is dropped on device.

The kernel is HBM-bandwidth bound (~320-360 GB/s/NC, the 8-core share of
the chip's HBM). The rel-err budget (2e-2) allows fp8: most of the
neighbor stream is cast to fp8_e3m4 (4 mantissa bits) on the host with
error-feedback rounding along the group axis (residual carried
group-to-group, absorbed by the final bf16 groups), which keeps
end-to-end max rel err at the bf16-baseline level (~4e-3) while cutting
HBM traffic ~45%.

fp8 costs compute: DVE runs 1-byte ops at 1x (no fp8 packing on TRN2),
so the 26-group reduction no longer fits on DVE alone in the shortened
DMA period. The reduction is split across engines per 128-node tile:
  - K_BF groups stay bf16 in the stream (DVE adds them at 2x; all DVE
    bf16 ops use flat contiguous slices - strided APs defeat 2x mode)
  - Q_POOL fp8 groups pair-add on GPSIMD (idle otherwise) in one flat
    batched op; its 3 partials merge in DVE's tree
  - P_PE fp8 groups are node-major matmul-accumulated (lhsT=I) into a
    PSUM f32 tile by the PE (warm under sustained load); DVE's merged
    partial is injected there as one more matmul
  - DVE pair-adds the rest fp8->bf16 + folds the partial tree
Then: ACT scaled-copy (1/26) PSUM->bf16, 4 PE transposes, ACT copy to
SBUF, PE GEMM vs bf16 W (f32 PSUM), ACT relu, bf16 store.

The emission is software-pipelined in 3 stages (load+DVE/Pool reduce @
tile i, PE accumulate @ i-1, transpose+GEMM+relu+store @ i-2) so the
in-order PE stream never blocks on the per-tile PE<->ACT ping-pong and
each PSUM accumulation group stays contiguous in the PE stream, and the
neighbor load is split into consumer-ordered slabs (pool | pe | dve |
bf16) so each engine's data lands just before it runs.

Host: fp8/bf16 packing + error feedback in numpy; group order arranged
so every engine operand is a flat contiguous slice; W pre-rearranged to
[p, c, o] so its load is contiguous; bf16 output upcast to f32.
"""

import os
import sys

import numpy as np
import ml_dtypes

for _p in ("/opt/trn_rl_repo", "/root/.axon_site/_ro/trn_rl_repo"):
    if os.path.isdir(_p) and _p not in sys.path:
        sys.path.insert(0, _p)

import concourse.bass as bass
import concourse.tile as tile
from concourse import bacc, mybir
from concourse.masks import make_identity

N, K, D, O = 16384, 25, 512, 1024
G = K + 1  # neigh groups + self
N_CORES = 8
P = 128  # nodes per tile (partition count)
INV = 1.0 / (K + 1)
FP = mybir.dt.float32
BF = mybir.dt.bfloat16
F8 = mybir.dt.float8e3
NP_BF = ml_dtypes.bfloat16
NP_F8 = ml_dtypes.float8_e3m4

# reduction split (groups): GPSIMD | PE | DVE fp8 | DVE bf16
Q_POOL = 6
P_PE = 10
K_BF = 4
R_DVE = G - Q_POOL - P_PE - K_BF  # 6
G8 = G - K_BF  # fp8 groups in the stream (22)
PEB = Q_POOL + P_PE  # end of PE slab (16)
# partial slots: 0-2 DVE fp8, 3-5 GPSIMD, 6-7 DVE bf16
NPART = R_DVE // 2 + Q_POOL // 2 + K_BF // 2  # 8
assert R_DVE % 2 == 0 and Q_POOL % 2 == 0 and K_BF % 2 == 0


def build_nc(n_nodes: int, neigh_bufs: int = 8) -> bass.Bass:
    """Build the per-core Bass program for a shard of `n_nodes` nodes."""
    assert n_nodes % P == 0
    nt = n_nodes // P

    nc = bacc.Bacc("TRN2", target_bir_lowering=False, debug=False)
    pk8_h = nc.dram_tensor("pk8", [n_nodes, G8 * D], F8, kind="ExternalInput")
    pkb_h = nc.dram_tensor("pkb", [n_nodes, K_BF * D], BF, kind="ExternalInput")
    # W pre-rearranged on host to [p, c, o] (row p = partition line of every
    # d-chunk's rhs) so the device load is contiguous 8KB rows
    w_h = nc.dram_tensor("W", [P, (D // P) * O], BF, kind="ExternalInput")
    out_h = nc.dram_tensor("out", [n_nodes, O], BF, kind="ExternalOutput")

    n_dc = D // P  # d-chunks for transposes / GEMM contraction
    n_oh = O // 512

    def g8(t, a, b):  # flat slice of D-wide group units [a, b)
        return t[:, a * D : b * D]

    with tile.TileContext(nc) as tc:
        with (
            tc.tile_pool(name="const", bufs=1) as const_pool,
            tc.tile_pool(name="neigh", bufs=neigh_bufs) as neigh_pool,
            tc.tile_pool(name="parts", bufs=3) as parts_pool,
            tc.tile_pool(name="small", bufs=3) as small_pool,
            tc.tile_pool(name="outp", bufs=3) as out_pool,
            tc.tile_pool(name="ps_a", bufs=2, space="PSUM") as ps_a_pool,
            tc.tile_pool(name="ps_t", bufs=2, space="PSUM") as ps_t_pool,
            tc.tile_pool(name="ps_o", bufs=2, space="PSUM") as ps_o_pool,
        ):
            w_sb = const_pool.tile([P, n_dc * O], BF)
            ident = const_pool.tile([P, P], BF)
            make_identity(nc, ident)
            # W rides the scalar hwdge queue, parallel to the neigh stream
            nc.scalar.dma_start(w_sb, w_h[:, :])

            nh8s, parts_l, psA_l, means_l = {}, {}, {}, {}

            def stage_load(i):
                nh8 = neigh_pool.tile([P, G8 * D], F8, tag="nh8", name="nh8")
                rows = bass.ts(i, P)
                # all loads on the sync queue: it carries no compute, so
                # triggers never queue behind waiting ACTIVATEs and the
                # prefetch runway is limited only by buffer depth
                nc.sync.dma_start(g8(nh8, 0, Q_POOL), pk8_h[rows, : Q_POOL * D])
                nc.sync.dma_start(
                    g8(nh8, Q_POOL, PEB), pk8_h[rows, Q_POOL * D : PEB * D]
                )
                nc.sync.dma_start(g8(nh8, PEB, G8), pk8_h[rows, PEB * D :])
                nhb = neigh_pool.tile([P, K_BF * D], BF, tag="nhb", name="nhb")
                nc.sync.dma_start(nhb, pkb_h[rows, :])
                nh8s[i] = (nh8, nhb)

            def stage_reduce(i):
                nh8, nhb = nh8s[i]
                parts = parts_pool.tile([P, NPART * D], BF, tag="pt", name="pt")
                parts_l[i] = parts
                # GPSIMD: groups [0,6) -> slots 5-7 (one flat batched op);
                # the PE absorbs these partials directly, so the DVE tree
                # never depends on GPSIMD
                nc.gpsimd.tensor_add(
                    g8(parts, 5, 8), g8(nh8, 0, 3), g8(nh8, 3, 6)
                )
                # DVE: fp8 pairs -> slots 0-2, bf16 pairs -> slots 3-4
                nc.vector.tensor_add(
                    g8(parts, 0, 3), g8(nh8, PEB, PEB + 3), g8(nh8, PEB + 3, G8)
                )
                nc.vector.tensor_add(
                    g8(parts, 3, 5), nhb[:, 0 : 2 * D], nhb[:, 2 * D : 4 * D]
                )
                # tree over DVE's own partials only: {0,1}+{2,3} -> {0}+{1}
                # -> {0}+{4}
                nc.vector.tensor_add(g8(parts, 0, 2), g8(parts, 0, 2), g8(parts, 2, 4))
                nc.vector.tensor_add(g8(parts, 0, 1), g8(parts, 0, 1), g8(parts, 1, 2))
                nc.vector.tensor_add(g8(parts, 0, 1), g8(parts, 0, 1), g8(parts, 4, 5))

            def stage_pe(i):
                # PE: node-major accumulate raw fp8 groups + DVE's merged
                # partial (lhsT=I) in one contiguous PSUM accumulation group
                # (runs one tile behind the DVE frontier, so no PE stall)
                nh8, _ = nh8s[i]
                psA = ps_a_pool.tile([P, D], FP, tag="psA", name="psA")
                psA_l[i] = psA
                for j in range(P_PE):
                    nc.tensor.matmul(
                        psA,
                        lhsT=ident,
                        rhs=g8(nh8, Q_POOL + j, Q_POOL + j + 1),
                        start=(j == 0),
                        stop=False,
                    )
                # inject DVE's merged partial + GPSIMD's three partials
                for s in (0, 5, 6, 7):
                    nc.tensor.matmul(
                        psA, lhsT=ident, rhs=g8(parts_l[i], s, s + 1),
                        start=False, stop=(s == 7),
                    )
                means = small_pool.tile([P, D], BF, tag="mn", name="mn")
                means_l[i] = means
                nc.scalar.activation(
                    means, psA, mybir.ActivationFunctionType.Copy, scale=INV
                )

            def stage_gemm(i):
                means = means_l.pop(i)
                sumT = small_pool.tile([P, D], BF, tag="tsb", name="tsb")
                tps = ps_t_pool.tile([P, D], BF, tag="tps", name="tps")
                for c in range(n_dc):
                    nc.tensor.transpose(
                        tps[:, bass.ts(c, P)], means[:, bass.ts(c, P)], ident
                    )
                nc.scalar.activation(sumT, tps, mybir.ActivationFunctionType.Copy)
                out_pss = [
                    ps_o_pool.tile([P, 512], FP, tag=f"ops{oh}", name=f"ops{oh}")
                    for oh in range(n_oh)
                ]
                for c in range(n_dc):
                    for oh in range(n_oh):
                        nc.tensor.matmul(
                            out_pss[oh],
                            lhsT=sumT[:, bass.ts(c, P)],
                            rhs=w_sb[:, c * O + oh * 512 : c * O + oh * 512 + 512],
                            start=(c == 0),
                            stop=(c == n_dc - 1),
                        )
                out_sb = out_pool.tile([P, O], BF)
                for oh in range(n_oh):
                    nc.scalar.activation(
                        out_sb[:, bass.ts(oh, 512)],
                        out_pss[oh],
                        mybir.ActivationFunctionType.Relu,
                    )
                    if i == nt - 1:
                        nc.scalar.dma_start(
                            out_h[bass.ts(i, P), bass.ts(oh, 512)],
                            out_sb[:, bass.ts(oh, 512)],
                        )
                if i != nt - 1:
                    nc.scalar.dma_start(out_h[bass.ts(i, P), :], out_sb)

            # 3-stage software pipeline: load+reduce @ i, PE accumulate @
            # i-1, transpose+GEMM @ i-2
            for i in range(nt + 2):
                if i < nt:
                    stage_load(i)
                    stage_reduce(i)
                if 1 <= i < nt + 1:
                    stage_pe(i - 1)
                if i >= 2:
                    stage_gemm(i - 2)

    nc.compile()
    return nc


def shard_inputs(inputs: dict) -> list[dict]:
    n = inputs["self_vecs"].shape[0]
    per = n // N_CORES
    # pack [neigh ; self] as [N, G, D] f32; quantize with error feedback
    # along the group axis: fp8 groups first (carrying the rounding
    # residual forward), bf16 groups last (absorbing the residual at
    # bf16 precision)
    x = np.empty((n, G, D), dtype=np.float32)
    x[:, :K] = inputs["neigh_vecs"]
    x[:, K] = inputs["self_vecs"]
    pk8 = np.empty((n, G8, D), dtype=NP_F8)
    pkb = np.empty((n, K_BF, D), dtype=NP_BF)
    e = np.zeros((n, D), dtype=np.float32)
    for g in range(G8):
        t = x[:, g] + e
        pk8[:, g] = t.astype(NP_F8)
        e = t - pk8[:, g].astype(np.float32)
    for g in range(K_BF):
        t = x[:, G8 + g] + e
        pkb[:, g] = t.astype(NP_BF)
        e = t - pkb[:, g].astype(np.float32)
    # pre-rearrange W to [p, c, o] so the device load is contiguous rows
    w_bf = np.ascontiguousarray(
        inputs["W"].reshape(D // P, P, O).transpose(1, 0, 2).reshape(P, -1),
        dtype=NP_BF,
    )
    pk8 = pk8.reshape(n, G8 * D)
    pkb = pkb.reshape(n, K_BF * D)
    maps = []
    for c in range(N_CORES):
        sl = slice(c * per, (c + 1) * per)
        maps.append({"pk8": pk8[sl], "pkb": pkb[sl], "W": w_bf})
    return maps


def run_sharded(inputs: dict, trace: bool = False, **kwargs):
    from concourse.bass_utils import run_bass_kernel_spmd

    in_maps = shard_inputs(inputs)
    n_nodes = in_maps[0]["pk8"].shape[0]
    nc = build_nc(n_nodes)
    res = run_bass_kernel_spmd(
        nc, in_maps, core_ids=list(range(N_CORES)), trace=trace, **kwargs
    )
    out = np.concatenate(
        [res.results[c]["out"] for c in range(N_CORES)], axis=0
    ).astype(np.float32)
    return out, res


def kernel(**inputs) -> np.ndarray:
    out, _ = run_sharded(inputs, trace=False)
    return out


# revision 18
# speedup vs baseline: 1.2840x; 1.0108x over previous
"""GCN aggregator kernel for Trainium2 (Bass/Tile), 8-core data-parallel.

Computes: out = relu(((sum_g x[:,g,:]) / (K+1)) @ W + b), x = [neigh;self]
Sharding: nodes (N) split evenly across 8 NeuronCores; W replicated. b is
zeros per the problem spec and is dropped on device.

The kernel is HBM-bandwidth bound (~320-360 GB/s/NC, the 8-core share of
the chip's HBM). The rel-err budget (2e-2) allows fp8: most of the
neighbor stream is cast to fp8_e3m4 (4 mantissa bits) on the host with
error-feedback rounding along the group axis (residual carried
group-to-group, absorbed by the final bf16 groups), which keeps
end-to-end max rel err at the bf16-baseline level (~4e-3) while cutting
HBM traffic ~45%.

fp8 costs compute: DVE runs 1-byte ops at 1x (no fp8 packing on TRN2),
so the 26-group reduction no longer fits on DVE alone in the shortened
DMA period. The reduction is split across engines per 128-node tile:
  - K_BF groups stay bf16 in the stream (DVE adds them at 2x; all DVE
    bf16 ops use flat contiguous slices - strided APs defeat 2x mode)
  - Q_POOL fp8 groups pair-add on GPSIMD (idle otherwise) in one flat
    batched op; its 3 partials merge in DVE's tree
  - P_PE fp8 groups are node-major matmul-accumulated (lhsT=I) into a
    PSUM f32 tile by the PE (warm under sustained load); DVE's merged
    partial is injected there as one more matmul
  - DVE pair-adds the rest fp8->bf16 + folds the partial tree
Then: ACT scaled-copy (1/26) PSUM->bf16, 4 PE transposes, ACT copy to
SBUF, PE GEMM vs bf16 W (f32 PSUM), ACT relu, bf16 store.

The emission is software-pipelined in 3 stages (load+DVE/Pool reduce @
tile i, PE accumulate @ i-1, transpose+GEMM+relu+store @ i-2) so the
in-order PE stream never blocks on the per-tile PE<->ACT ping-pong and
each PSUM accumulation group stays contiguous in the PE stream, and the
neighbor load is split into consumer-ordered slabs (pool | pe | dve |
bf16) so each engine's data lands just before it runs.

Host: fp8/bf16 packing + error feedback in numpy; group order arranged
so every engine operand is a flat contiguous slice; W pre-rearranged to
[p, c, o] so its load is contiguous; bf16 output upcast to f32.
"""

import os
import sys

import numpy as np
import ml_dtypes

for _p in ("/opt/trn_rl_repo", "/root/.axon_site/_ro/trn_rl_repo"):
    if os.path.isdir(_p) and _p not in sys.path:
        sys.path.insert(0, _p)

import concourse.bass as bass
import concourse.tile as tile
from concourse import bacc, mybir
from concourse.masks import make_identity

N, K, D, O = 16384, 25, 512, 1024
G = K + 1  # neigh groups + self
N_CORES = 8
P = 128  # nodes per tile (partition count)
INV = 1.0 / (K + 1)
FP = mybir.dt.float32
BF = mybir.dt.bfloat16
F8 = mybir.dt.float8e3
NP_BF = ml_dtypes.bfloat16
NP_F8 = ml_dtypes.float8_e3m4

# reduction split (groups): GPSIMD | PE | DVE fp8 | DVE bf16
Q_POOL = 6
P_PE = 10
K_BF = 4
R_DVE = G - Q_POOL - P_PE - K_BF  # 6
G8 = G - K_BF  # fp8 groups in the stream (22)
PEB = Q_POOL + P_PE  # end of PE slab (16)
# partial slots: 0-2 DVE fp8, 3-5 GPSIMD, 6-7 DVE bf16
NPART = R_DVE // 2 + Q_POOL // 2 + K_BF // 2  # 8
assert R_DVE % 2 == 0 and Q_POOL % 2 == 0 and K_BF % 2 == 0


def build_nc(n_nodes: int, neigh_bufs: int = 8) -> bass.Bass:
    """Build the per-core Bass program for a shard of `n_nodes` nodes."""
    assert n_nodes % P == 0
    nt = n_nodes // P

    nc = bacc.Bacc("TRN2", target_bir_lowering=False, debug=False)
    pk8_h = nc.dram_tensor("pk8", [n_nodes, G8 * D], F8, kind="ExternalInput")
    pkb_h = nc.dram_tensor("pkb", [n_nodes, K_BF * D], BF, kind="ExternalInput")
    # W pre-rearranged on host to [p, c, o] (row p = partition line of every
    # d-chunk's rhs) so the device load is contiguous 8KB rows
    w_h = nc.dram_tensor("W", [P, (D // P) * O], BF, kind="ExternalInput")
    out_h = nc.dram_tensor("out", [n_nodes, O], BF, kind="ExternalOutput")

    n_dc = D // P  # d-chunks for transposes / GEMM contraction
    n_oh = O // 512

    def g8(t, a, b):  # flat slice of D-wide group units [a, b)
        return t[:, a * D : b * D]

    with tile.TileContext(nc) as tc:
        with (
            tc.tile_pool(name="const", bufs=1) as const_pool,
            tc.tile_pool(name="neigh", bufs=neigh_bufs) as neigh_pool,
            tc.tile_pool(name="parts", bufs=3) as parts_pool,
            tc.tile_pool(name="small", bufs=3) as small_pool,
            tc.tile_pool(name="outp", bufs=3) as out_pool,
            tc.tile_pool(name="ps_a", bufs=2, space="PSUM") as ps_a_pool,
            tc.tile_pool(name="ps_t", bufs=2, space="PSUM") as ps_t_pool,
            tc.tile_pool(name="ps_o", bufs=2, space="PSUM") as ps_o_pool,
        ):
            w_sb = const_pool.tile([P, n_dc * O], BF)
            ident = const_pool.tile([P, P], BF)
            make_identity(nc, ident)
            # W rides the scalar hwdge queue, parallel to the neigh stream
            nc.scalar.dma_start(w_sb, w_h[:, :])

            nh8s, parts_l, psA_l, means_l = {}, {}, {}, {}

            def stage_load(i):
                nh8 = neigh_pool.tile([P, G8 * D], F8, tag="nh8", name="nh8")
                rows = bass.ts(i, P)
                # all loads on the sync queue (no compute -> triggers never
                # queue behind waiting ACTIVATEs), ordered for the tightest
                # consumer first: DVE slabs, then pool (5us slack) and the
                # PE slab (consumed one tile later)
                nhb = neigh_pool.tile([P, K_BF * D], BF, tag="nhb", name="nhb")
                nc.sync.dma_start(g8(nh8, PEB, G8), pk8_h[rows, PEB * D :])
                nc.sync.dma_start(nhb, pkb_h[rows, :])
                nc.sync.dma_start(g8(nh8, 0, Q_POOL), pk8_h[rows, : Q_POOL * D])
                nc.sync.dma_start(
                    g8(nh8, Q_POOL, PEB), pk8_h[rows, Q_POOL * D : PEB * D]
                )
                nh8s[i] = (nh8, nhb)

            def stage_reduce(i):
                nh8, nhb = nh8s[i]
                parts = parts_pool.tile([P, NPART * D], BF, tag="pt", name="pt")
                parts_l[i] = parts
                # GPSIMD: groups [0,6) -> slots 5-7 (one flat batched op);
                # the PE absorbs these partials directly, so the DVE tree
                # never depends on GPSIMD
                nc.gpsimd.tensor_add(
                    g8(parts, 5, 8), g8(nh8, 0, 3), g8(nh8, 3, 6)
                )
                # DVE: fp8 pairs -> slots 0-2, bf16 pairs -> slots 3-4
                nc.vector.tensor_add(
                    g8(parts, 0, 3), g8(nh8, PEB, PEB + 3), g8(nh8, PEB + 3, G8)
                )
                nc.vector.tensor_add(
                    g8(parts, 3, 5), nhb[:, 0 : 2 * D], nhb[:, 2 * D : 4 * D]
                )
                # tree over DVE's own partials only: {0,1}+{2,3} -> {0}+{1}
                # -> {0}+{4}
                nc.vector.tensor_add(g8(parts, 0, 2), g8(parts, 0, 2), g8(parts, 2, 4))
                nc.vector.tensor_add(g8(parts, 0, 1), g8(parts, 0, 1), g8(parts, 1, 2))
                nc.vector.tensor_add(g8(parts, 0, 1), g8(parts, 0, 1), g8(parts, 4, 5))

            def stage_pe(i):
                # PE: node-major accumulate raw fp8 groups + DVE's merged
                # partial (lhsT=I) in one contiguous PSUM accumulation group
                # (runs one tile behind the DVE frontier, so no PE stall)
                nh8, _ = nh8s[i]
                psA = ps_a_pool.tile([P, D], FP, tag="psA", name="psA")
                psA_l[i] = psA
                for j in range(P_PE):
                    nc.tensor.matmul(
                        psA,
                        lhsT=ident,
                        rhs=g8(nh8, Q_POOL + j, Q_POOL + j + 1),
                        start=(j == 0),
                        stop=False,
                    )
                # inject DVE's merged partial + GPSIMD's three partials
                for s in (0, 5, 6, 7):
                    nc.tensor.matmul(
                        psA, lhsT=ident, rhs=g8(parts_l[i], s, s + 1),
                        start=False, stop=(s == 7),
                    )
                means = small_pool.tile([P, D], BF, tag="mn", name="mn")
                means_l[i] = means
                nc.scalar.activation(
                    means, psA, mybir.ActivationFunctionType.Copy, scale=INV
                )

            def stage_gemm(i):
                means = means_l.pop(i)
                sumT = small_pool.tile([P, D], BF, tag="tsb", name="tsb")
                tps = ps_t_pool.tile([P, D], BF, tag="tps", name="tps")
                for c in range(n_dc):
                    nc.tensor.transpose(
                        tps[:, bass.ts(c, P)], means[:, bass.ts(c, P)], ident
                    )
                nc.scalar.activation(sumT, tps, mybir.ActivationFunctionType.Copy)
                out_pss = [
                    ps_o_pool.tile([P, 512], FP, tag=f"ops{oh}", name=f"ops{oh}")
                    for oh in range(n_oh)
                ]
                for c in range(n_dc):
                    for oh in range(n_oh):
                        nc.tensor.matmul(
                            out_pss[oh],
                            lhsT=sumT[:, bass.ts(c, P)],
                            rhs=w_sb[:, c * O + oh * 512 : c * O + oh * 512 + 512],
                            start=(c == 0),
                            stop=(c == n_dc - 1),
                        )
                out_sb = out_pool.tile([P, O], BF)
                for oh in range(n_oh):
                    nc.scalar.activation(
                        out_sb[:, bass.ts(oh, 512)],
                        out_pss[oh],
                        mybir.ActivationFunctionType.Relu,
                    )
                    if i == nt - 1:
                        nc.scalar.dma_start(
                            out_h[bass.ts(i, P), bass.ts(oh, 512)],
                            out_sb[:, bass.ts(oh, 512)],
                        )
                if i != nt - 1:
                    nc.scalar.dma_start(out_h[bass.ts(i, P), :], out_sb)

            # 3-stage software pipeline: load+reduce @ i, PE accumulate @
            # i-1, transpose+GEMM @ i-2
            for i in range(nt + 2):
                if i < nt:
                    stage_load(i)
                    stage_reduce(i)
                if 1 <= i < nt + 1:
                    stage_pe(i - 1)
                if i >= 2:
                    stage_gemm(i - 2)

    nc.compile()
    return nc


def shard_inputs(inputs: dict) -> list[dict]:
    n = inputs["self_vecs"].shape[0]
    per = n // N_CORES
    # pack [neigh ; self] as [N, G, D] f32; quantize with error feedback
    # along the group axis: fp8 groups first (carrying the rounding
    # residual forward), bf16 groups last (absorbing the residual at
    # bf16 precision)
    x = np.empty((n, G, D), dtype=np.float32)
    x[:, :K] = inputs["neigh_vecs"]
    x[:, K] = inputs["self_vecs"]
    pk8 = np.empty((n, G8, D), dtype=NP_F8)
    pkb = np.empty((n, K_BF, D), dtype=NP_BF)
    e = np.zeros((n, D), dtype=np.float32)
    for g in range(G8):
        t = x[:, g] + e
        pk8[:, g] = t.astype(NP_F8)
        e = t - pk8[:, g].astype(np.float32)
    for g in range(K_BF):
        t = x[:, G8 + g] + e
        pkb[:, g] = t.astype(NP_BF)
        e = t - pkb[:, g].astype(np.float32)
    # pre-rearrange W to [p, c, o] so the device load is contiguous rows
    w_bf = np.ascontiguousarray(
        inputs["W"].reshape(D // P, P, O).transpose(1, 0, 2).reshape(P, -1),
        dtype=NP_BF,
    )
    pk8 = pk8.reshape(n, G8 * D)
    pkb = pkb.reshape(n, K_BF * D)
    maps = []
    for c in range(N_CORES):
        sl = slice(c * per, (c + 1) * per)
        maps.append({"pk8": pk8[sl], "pkb": pkb[sl], "W": w_bf})
    return maps


def run_sharded(inputs: dict, trace: bool = False, **kwargs):
    from concourse.bass_utils import run_bass_kernel_spmd

    in_maps = shard_inputs(inputs)
    n_nodes = in_maps[0]["pk8"].shape[0]
    nc = build_nc(n_nodes)
    res = run_bass_kernel_spmd(
        nc, in_maps, core_ids=list(range(N_CORES)), trace=trace, **kwargs
    )
    out = np.concatenate(
        [res.results[c]["out"] for c in range(N_CORES)], axis=0
    ).astype(np.float32)
    return out, res


def kernel(**inputs) -> np.ndarray:
    out, _ = run_sharded(inputs, trace=False)
    return out


# revision 19
# speedup vs baseline: 1.3032x; 1.0150x over previous
"""GCN aggregator kernel for Trainium2 (Bass/Tile), 8-core data-parallel.

Computes: out = relu(((sum_g x[:,g,:]) / (K+1)) @ W + b), x = [neigh;self]
Sharding: nodes (N) split evenly across 8 NeuronCores; W replicated. b is
zeros per the problem spec and is dropped on device.

The kernel is HBM-bandwidth bound (~320-360 GB/s/NC, the 8-core share of
the chip's HBM). The rel-err budget (2e-2) allows fp8: most of the
neighbor stream is cast to fp8_e3m4 (4 mantissa bits) on the host with
error-feedback rounding along the group axis (residual carried
group-to-group, absorbed by the final bf16 groups), which keeps
end-to-end max rel err at the bf16-baseline level (~4e-3) while cutting
HBM traffic ~45%.

fp8 costs compute: DVE runs 1-byte ops at 1x (no fp8 packing on TRN2),
so the 26-group reduction no longer fits on DVE alone in the shortened
DMA period. The reduction is split across engines per 128-node tile:
  - K_BF groups stay bf16 in the stream (DVE adds them at 2x; all DVE
    bf16 ops use flat contiguous slices - strided APs defeat 2x mode)
  - Q_POOL fp8 groups pair-add on GPSIMD (idle otherwise) in one flat
    batched op; its 3 partials merge in DVE's tree
  - P_PE fp8 groups are node-major matmul-accumulated (lhsT=I) into a
    PSUM f32 tile by the PE (warm under sustained load); DVE's merged
    partial is injected there as one more matmul
  - DVE pair-adds the rest fp8->bf16 + folds the partial tree
Then: ACT scaled-copy (1/26) PSUM->bf16, 4 PE transposes, ACT copy to
SBUF, PE GEMM vs bf16 W (f32 PSUM), ACT relu, bf16 store.

The emission is software-pipelined in 3 stages (load+DVE/Pool reduce @
tile i, PE accumulate @ i-1, transpose+GEMM+relu+store @ i-2) so the
in-order PE stream never blocks on the per-tile PE<->ACT ping-pong and
each PSUM accumulation group stays contiguous in the PE stream, and the
neighbor load is split into consumer-ordered slabs (pool | pe | dve |
bf16) so each engine's data lands just before it runs.

Host: fp8/bf16 packing + error feedback in numpy; group order arranged
so every engine operand is a flat contiguous slice; W pre-rearranged to
[p, c, o] so its load is contiguous; bf16 output upcast to f32.
"""

import os
import sys

import numpy as np
import ml_dtypes

for _p in ("/opt/trn_rl_repo", "/root/.axon_site/_ro/trn_rl_repo"):
    if os.path.isdir(_p) and _p not in sys.path:
        sys.path.insert(0, _p)

import concourse.bass as bass
import concourse.tile as tile
from concourse import bacc, mybir
from concourse.masks import make_identity

N, K, D, O = 16384, 25, 512, 1024
G = K + 1  # neigh groups + self
N_CORES = 8
P = 128  # nodes per tile (partition count)
INV = 1.0 / (K + 1)
FP = mybir.dt.float32
BF = mybir.dt.bfloat16
F8 = mybir.dt.float8e3
NP_BF = ml_dtypes.bfloat16
NP_F8 = ml_dtypes.float8_e3m4

# reduction split (groups): GPSIMD | PE | DVE fp8 | DVE bf16
Q_POOL = 6
P_PE = 12
K_BF = 4
R_DVE = G - Q_POOL - P_PE - K_BF  # 4
G8 = G - K_BF  # fp8 groups in the stream (22)
PEB = Q_POOL + P_PE  # end of PE slab (18)
# partial slots: 0-1 DVE fp8, 2-3 DVE bf16, 4-6 GPSIMD
NPART = R_DVE // 2 + Q_POOL // 2 + K_BF // 2  # 7
assert R_DVE % 2 == 0 and Q_POOL % 2 == 0 and K_BF % 2 == 0


def build_nc(n_nodes: int, neigh_bufs: int = 8) -> bass.Bass:
    """Build the per-core Bass program for a shard of `n_nodes` nodes."""
    assert n_nodes % P == 0
    nt = n_nodes // P

    nc = bacc.Bacc("TRN2", target_bir_lowering=False, debug=False)
    pk8_h = nc.dram_tensor("pk8", [n_nodes, G8 * D], F8, kind="ExternalInput")
    pkb_h = nc.dram_tensor("pkb", [n_nodes, K_BF * D], BF, kind="ExternalInput")
    # W pre-rearranged on host to [p, c, o] (row p = partition line of every
    # d-chunk's rhs) so the device load is contiguous 8KB rows
    w_h = nc.dram_tensor("W", [P, (D // P) * O], BF, kind="ExternalInput")
    out_h = nc.dram_tensor("out", [n_nodes, O], BF, kind="ExternalOutput")

    n_dc = D // P  # d-chunks for transposes / GEMM contraction
    n_oh = O // 512

    def g8(t, a, b):  # flat slice of D-wide group units [a, b)
        return t[:, a * D : b * D]

    with tile.TileContext(nc) as tc:
        with (
            tc.tile_pool(name="const", bufs=1) as const_pool,
            tc.tile_pool(name="neigh", bufs=neigh_bufs) as neigh_pool,
            tc.tile_pool(name="parts", bufs=3) as parts_pool,
            tc.tile_pool(name="small", bufs=3) as small_pool,
            tc.tile_pool(name="outp", bufs=3) as out_pool,
            tc.tile_pool(name="ps_a", bufs=2, space="PSUM") as ps_a_pool,
            tc.tile_pool(name="ps_t", bufs=2, space="PSUM") as ps_t_pool,
            tc.tile_pool(name="ps_o", bufs=2, space="PSUM") as ps_o_pool,
        ):
            w_sb = const_pool.tile([P, n_dc * O], BF)
            ident = const_pool.tile([P, P], BF)
            make_identity(nc, ident)
            # W rides the scalar hwdge queue, parallel to the neigh stream
            nc.scalar.dma_start(w_sb, w_h[:, :])

            nh8s, parts_l, psA_l, means_l = {}, {}, {}, {}

            def stage_load(i):
                nh8 = neigh_pool.tile([P, G8 * D], F8, tag="nh8", name="nh8")
                rows = bass.ts(i, P)
                # all loads on the sync queue (no compute -> triggers never
                # queue behind waiting ACTIVATEs), ordered for the tightest
                # consumer first: DVE slabs, then pool (5us slack) and the
                # PE slab (consumed one tile later)
                nhb = neigh_pool.tile([P, K_BF * D], BF, tag="nhb", name="nhb")
                nc.sync.dma_start(g8(nh8, PEB, G8), pk8_h[rows, PEB * D :])
                nc.sync.dma_start(nhb, pkb_h[rows, :])
                nc.sync.dma_start(g8(nh8, 0, Q_POOL), pk8_h[rows, : Q_POOL * D])
                nc.sync.dma_start(
                    g8(nh8, Q_POOL, PEB), pk8_h[rows, Q_POOL * D : PEB * D]
                )
                nh8s[i] = (nh8, nhb)

            def stage_reduce(i):
                nh8, nhb = nh8s[i]
                parts = parts_pool.tile([P, NPART * D], BF, tag="pt", name="pt")
                parts_l[i] = parts
                # GPSIMD: groups [0,6) -> slots 5-7 (one flat batched op);
                # the PE absorbs these partials directly, so the DVE tree
                # never depends on GPSIMD
                nc.gpsimd.tensor_add(
                    g8(parts, 4, 7), g8(nh8, 0, 3), g8(nh8, 3, 6)
                )
                # DVE: fp8 pairs -> slots 0-1, bf16 pairs -> slots 2-3
                nc.vector.tensor_add(
                    g8(parts, 0, 2), g8(nh8, PEB, PEB + 2), g8(nh8, PEB + 2, G8)
                )
                nc.vector.tensor_add(
                    g8(parts, 2, 4), nhb[:, 0 : 2 * D], nhb[:, 2 * D : 4 * D]
                )
                # tree over DVE's own partials only: {0,1}+{2,3} -> {0}+{1}
                nc.vector.tensor_add(g8(parts, 0, 2), g8(parts, 0, 2), g8(parts, 2, 4))
                nc.vector.tensor_add(g8(parts, 0, 1), g8(parts, 0, 1), g8(parts, 1, 2))

            def stage_pe(i):
                # PE: node-major accumulate raw fp8 groups + DVE's merged
                # partial (lhsT=I) in one contiguous PSUM accumulation group
                # (runs one tile behind the DVE frontier, so no PE stall)
                nh8, _ = nh8s[i]
                psA = ps_a_pool.tile([P, D], FP, tag="psA", name="psA")
                psA_l[i] = psA
                for j in range(P_PE):
                    nc.tensor.matmul(
                        psA,
                        lhsT=ident,
                        rhs=g8(nh8, Q_POOL + j, Q_POOL + j + 1),
                        start=(j == 0),
                        stop=False,
                    )
                # inject DVE's merged partial + GPSIMD's three partials
                for s in (0, 4, 5, 6):
                    nc.tensor.matmul(
                        psA, lhsT=ident, rhs=g8(parts_l[i], s, s + 1),
                        start=False, stop=(s == 6),
                    )
                means = small_pool.tile([P, D], BF, tag="mn", name="mn")
                means_l[i] = means
                nc.scalar.activation(
                    means, psA, mybir.ActivationFunctionType.Copy, scale=INV
                )

            def stage_gemm(i):
                means = means_l.pop(i)
                sumT = small_pool.tile([P, D], BF, tag="tsb", name="tsb")
                tps = ps_t_pool.tile([P, D], BF, tag="tps", name="tps")
                for c in range(n_dc):
                    nc.tensor.transpose(
                        tps[:, bass.ts(c, P)], means[:, bass.ts(c, P)], ident
                    )
                nc.scalar.activation(sumT, tps, mybir.ActivationFunctionType.Copy)
                out_pss = [
                    ps_o_pool.tile([P, 512], FP, tag=f"ops{oh}", name=f"ops{oh}")
                    for oh in range(n_oh)
                ]
                for c in range(n_dc):
                    for oh in range(n_oh):
                        nc.tensor.matmul(
                            out_pss[oh],
                            lhsT=sumT[:, bass.ts(c, P)],
                            rhs=w_sb[:, c * O + oh * 512 : c * O + oh * 512 + 512],
                            start=(c == 0),
                            stop=(c == n_dc - 1),
                        )
                out_sb = out_pool.tile([P, O], BF)
                for oh in range(n_oh):
                    nc.scalar.activation(
                        out_sb[:, bass.ts(oh, 512)],
                        out_pss[oh],
                        mybir.ActivationFunctionType.Relu,
                    )
                    if i == nt - 1:
                        nc.scalar.dma_start(
                            out_h[bass.ts(i, P), bass.ts(oh, 512)],
                            out_sb[:, bass.ts(oh, 512)],
                        )
                if i != nt - 1:
                    nc.scalar.dma_start(out_h[bass.ts(i, P), :], out_sb)

            # 3-stage software pipeline: load+reduce @ i, PE accumulate @
            # i-1, transpose+GEMM @ i-2
            for i in range(nt + 2):
                if i < nt:
                    stage_load(i)
                    stage_reduce(i)
                if 1 <= i < nt + 1:
                    stage_pe(i - 1)
                if i >= 2:
                    stage_gemm(i - 2)

    nc.compile()
    return nc


def shard_inputs(inputs: dict) -> list[dict]:
    n = inputs["self_vecs"].shape[0]
    per = n // N_CORES
    # pack [neigh ; self] as [N, G, D] f32; quantize with error feedback
    # along the group axis: fp8 groups first (carrying the rounding
    # residual forward), bf16 groups last (absorbing the residual at
    # bf16 precision)
    x = np.empty((n, G, D), dtype=np.float32)
    x[:, :K] = inputs["neigh_vecs"]
    x[:, K] = inputs["self_vecs"]
    pk8 = np.empty((n, G8, D), dtype=NP_F8)
    pkb = np.empty((n, K_BF, D), dtype=NP_BF)
    e = np.zeros((n, D), dtype=np.float32)
    for g in range(G8):
        t = x[:, g] + e
        pk8[:, g] = t.astype(NP_F8)
        e = t - pk8[:, g].astype(np.float32)
    for g in range(K_BF):
        t = x[:, G8 + g] + e
        pkb[:, g] = t.astype(NP_BF)
        e = t - pkb[:, g].astype(np.float32)
    # pre-rearrange W to [p, c, o] so the device load is contiguous rows
    w_bf = np.ascontiguousarray(
        inputs["W"].reshape(D // P, P, O).transpose(1, 0, 2).reshape(P, -1),
        dtype=NP_BF,
    )
    pk8 = pk8.reshape(n, G8 * D)
    pkb = pkb.reshape(n, K_BF * D)
    maps = []
    for c in range(N_CORES):
        sl = slice(c * per, (c + 1) * per)
        maps.append({"pk8": pk8[sl], "pkb": pkb[sl], "W": w_bf})
    return maps


def run_sharded(inputs: dict, trace: bool = False, **kwargs):
    from concourse.bass_utils import run_bass_kernel_spmd

    in_maps = shard_inputs(inputs)
    n_nodes = in_maps[0]["pk8"].shape[0]
    nc = build_nc(n_nodes)
    res = run_bass_kernel_spmd(
        nc, in_maps, core_ids=list(range(N_CORES)), trace=trace, **kwargs
    )
    out = np.concatenate(
        [res.results[c]["out"] for c in range(N_CORES)], axis=0
    ).astype(np.float32)
    return out, res


def kernel(**inputs) -> np.ndarray:
    out, _ = run_sharded(inputs, trace=False)
    return out


# revision 20
# speedup vs baseline: 1.3850x; 1.0628x over previous
"""GCN aggregator kernel for Trainium2 (Bass/Tile), 8-core data-parallel.

Computes: out = relu(((sum_g x[:,g,:]) / (K+1)) @ W + b), x = [neigh;self]
Sharding: nodes (N) split evenly across 8 NeuronCores; W replicated. b is
zeros per the problem spec and is dropped on device.

The kernel is HBM-bandwidth bound (~320-360 GB/s/NC, the 8-core share of
the chip's HBM). The rel-err budget (2e-2) allows fp8: most of the
neighbor stream is cast to fp8_e3m4 (4 mantissa bits) on the host with
error-feedback rounding along the group axis (residual carried
group-to-group, absorbed by the final bf16 groups), which keeps
end-to-end max rel err at the bf16-baseline level (~4e-3) while cutting
HBM traffic ~45%.

fp8 costs compute: DVE runs 1-byte ops at 1x (no fp8 packing on TRN2),
so the 26-group reduction no longer fits on DVE alone in the shortened
DMA period. The reduction is split across engines per 128-node tile:
  - K_BF groups stay bf16 in the stream (DVE adds them at 2x; all DVE
    bf16 ops use flat contiguous slices - strided APs defeat 2x mode)
  - Q_POOL fp8 groups pair-add on GPSIMD (idle otherwise) in one flat
    batched op; its 3 partials merge in DVE's tree
  - P_PE fp8 groups are node-major matmul-accumulated (lhsT=I) into a
    PSUM f32 tile by the PE (warm under sustained load); DVE's merged
    partial is injected there as one more matmul
  - DVE pair-adds the rest fp8->bf16 + folds the partial tree
Then: ACT scaled-copy (1/26) PSUM->bf16, 4 PE transposes, ACT copy to
SBUF, PE GEMM vs bf16 W (f32 PSUM), ACT relu, bf16 store.

The emission is software-pipelined in 3 stages (load+DVE/Pool reduce @
tile i, PE accumulate @ i-1, transpose+GEMM+relu+store @ i-2) so the
in-order PE stream never blocks on the per-tile PE<->ACT ping-pong and
each PSUM accumulation group stays contiguous in the PE stream, and the
neighbor load is split into consumer-ordered slabs (pool | pe | dve |
bf16) so each engine's data lands just before it runs.

Host: fp8/bf16 packing + error feedback in numpy; group order arranged
so every engine operand is a flat contiguous slice; W pre-rearranged to
[p, c, o] so its load is contiguous; bf16 output upcast to f32.
"""

import os
import sys

import numpy as np
import ml_dtypes

for _p in ("/opt/trn_rl_repo", "/root/.axon_site/_ro/trn_rl_repo"):
    if os.path.isdir(_p) and _p not in sys.path:
        sys.path.insert(0, _p)

import concourse.bass as bass
import concourse.tile as tile
from concourse import bacc, mybir
from concourse.masks import make_identity

N, K, D, O = 16384, 25, 512, 1024
G = K + 1  # neigh groups + self
N_CORES = 8
P = 128  # nodes per tile (partition count)
INV = 1.0 / (K + 1)
FP = mybir.dt.float32
BF = mybir.dt.bfloat16
F8 = mybir.dt.float8e3
NP_BF = ml_dtypes.bfloat16
NP_F8 = ml_dtypes.float8_e3m4

# reduction split (groups): GPSIMD | PE | DVE fp8 | DVE bf16
Q_POOL = 6
P_PE = 12
K_BF = 2
R_DVE = G - Q_POOL - P_PE - K_BF  # 6
G8 = G - K_BF  # fp8 groups in the stream (24)
PEB = Q_POOL + P_PE  # end of PE slab (18)
# partial slots: 0-2 DVE fp8, 3 DVE bf16, 4-6 GPSIMD
NPART = R_DVE // 2 + Q_POOL // 2 + K_BF // 2  # 7
assert R_DVE % 2 == 0 and Q_POOL % 2 == 0 and K_BF % 2 == 0


def build_nc(n_nodes: int, neigh_bufs: int = 8) -> bass.Bass:
    """Build the per-core Bass program for a shard of `n_nodes` nodes."""
    assert n_nodes % P == 0
    nt = n_nodes // P

    nc = bacc.Bacc("TRN2", target_bir_lowering=False, debug=False)
    pk8_h = nc.dram_tensor("pk8", [n_nodes, G8 * D], F8, kind="ExternalInput")
    pkb_h = nc.dram_tensor("pkb", [n_nodes, K_BF * D], BF, kind="ExternalInput")
    # W pre-rearranged on host to [p, c, o] (row p = partition line of every
    # d-chunk's rhs) so the device load is contiguous 8KB rows
    w_h = nc.dram_tensor("W", [P, (D // P) * O], BF, kind="ExternalInput")
    out_h = nc.dram_tensor("out", [n_nodes, O], BF, kind="ExternalOutput")

    n_dc = D // P  # d-chunks for transposes / GEMM contraction
    n_oh = O // 512

    def g8(t, a, b):  # flat slice of D-wide group units [a, b)
        return t[:, a * D : b * D]

    with tile.TileContext(nc) as tc:
        with (
            tc.tile_pool(name="const", bufs=1) as const_pool,
            tc.tile_pool(name="neigh", bufs=neigh_bufs) as neigh_pool,
            tc.tile_pool(name="parts", bufs=3) as parts_pool,
            tc.tile_pool(name="small", bufs=3) as small_pool,
            tc.tile_pool(name="outp", bufs=3) as out_pool,
            tc.tile_pool(name="ps_a", bufs=2, space="PSUM") as ps_a_pool,
            tc.tile_pool(name="ps_t", bufs=2, space="PSUM") as ps_t_pool,
            tc.tile_pool(name="ps_o", bufs=2, space="PSUM") as ps_o_pool,
        ):
            w_sb = const_pool.tile([P, n_dc * O], BF)
            ident = const_pool.tile([P, P], BF)
            make_identity(nc, ident)
            # W rides the scalar hwdge queue, parallel to the neigh stream
            nc.scalar.dma_start(w_sb, w_h[:, :])

            nh8s, parts_l, psA_l, means_l = {}, {}, {}, {}

            def stage_load(i):
                nh8 = neigh_pool.tile([P, G8 * D], F8, tag="nh8", name="nh8")
                rows = bass.ts(i, P)
                # all loads on the sync queue (no compute -> triggers never
                # queue behind waiting ACTIVATEs), ordered for the tightest
                # consumer first: DVE slabs, then pool (5us slack) and the
                # PE slab (consumed one tile later)
                nhb = neigh_pool.tile([P, K_BF * D], BF, tag="nhb", name="nhb")
                nc.sync.dma_start(g8(nh8, PEB, G8), pk8_h[rows, PEB * D :])
                nc.sync.dma_start(nhb, pkb_h[rows, :])
                nc.sync.dma_start(g8(nh8, 0, Q_POOL), pk8_h[rows, : Q_POOL * D])
                nc.sync.dma_start(
                    g8(nh8, Q_POOL, PEB), pk8_h[rows, Q_POOL * D : PEB * D]
                )
                nh8s[i] = (nh8, nhb)

            def stage_reduce(i):
                nh8, nhb = nh8s[i]
                parts = parts_pool.tile([P, NPART * D], BF, tag="pt", name="pt")
                parts_l[i] = parts
                # GPSIMD: groups [0,6) -> slots 5-7 (one flat batched op);
                # the PE absorbs these partials directly, so the DVE tree
                # never depends on GPSIMD
                nc.gpsimd.tensor_add(
                    g8(parts, 4, 7), g8(nh8, 0, 3), g8(nh8, 3, 6)
                )
                # DVE: fp8 pairs -> slots 0-2, bf16 pair -> slot 3
                nc.vector.tensor_add(
                    g8(parts, 0, 3), g8(nh8, PEB, PEB + 3), g8(nh8, PEB + 3, G8)
                )
                nc.vector.tensor_add(
                    g8(parts, 3, 4), nhb[:, 0:D], nhb[:, D : 2 * D]
                )
                # tree over DVE's own partials only: {0,1}+{2,3} -> {0}+{1}
                nc.vector.tensor_add(g8(parts, 0, 2), g8(parts, 0, 2), g8(parts, 2, 4))
                nc.vector.tensor_add(g8(parts, 0, 1), g8(parts, 0, 1), g8(parts, 1, 2))

            def stage_pe(i):
                # PE: node-major accumulate raw fp8 groups + DVE's merged
                # partial (lhsT=I) in one contiguous PSUM accumulation group
                # (runs one tile behind the DVE frontier, so no PE stall)
                nh8, _ = nh8s[i]
                psA = ps_a_pool.tile([P, D], FP, tag="psA", name="psA")
                psA_l[i] = psA
                for j in range(P_PE):
                    nc.tensor.matmul(
                        psA,
                        lhsT=ident,
                        rhs=g8(nh8, Q_POOL + j, Q_POOL + j + 1),
                        start=(j == 0),
                        stop=False,
                    )
                # inject DVE's merged partial + GPSIMD's three partials
                for s in (0, 4, 5, 6):
                    nc.tensor.matmul(
                        psA, lhsT=ident, rhs=g8(parts_l[i], s, s + 1),
                        start=False, stop=(s == 6),
                    )
                means = small_pool.tile([P, D], BF, tag="mn", name="mn")
                means_l[i] = means
                nc.scalar.activation(
                    means, psA, mybir.ActivationFunctionType.Copy, scale=INV
                )

            def stage_gemm(i):
                means = means_l.pop(i)
                sumT = small_pool.tile([P, D], BF, tag="tsb", name="tsb")
                tps = ps_t_pool.tile([P, D], BF, tag="tps", name="tps")
                for c in range(n_dc):
                    nc.tensor.transpose(
                        tps[:, bass.ts(c, P)], means[:, bass.ts(c, P)], ident
                    )
                nc.scalar.activation(sumT, tps, mybir.ActivationFunctionType.Copy)
                out_pss = [
                    ps_o_pool.tile([P, 512], FP, tag=f"ops{oh}", name=f"ops{oh}")
                    for oh in range(n_oh)
                ]
                for c in range(n_dc):
                    for oh in range(n_oh):
                        nc.tensor.matmul(
                            out_pss[oh],
                            lhsT=sumT[:, bass.ts(c, P)],
                            rhs=w_sb[:, c * O + oh * 512 : c * O + oh * 512 + 512],
                            start=(c == 0),
                            stop=(c == n_dc - 1),
                        )
                out_sb = out_pool.tile([P, O], BF)
                for oh in range(n_oh):
                    nc.scalar.activation(
                        out_sb[:, bass.ts(oh, 512)],
                        out_pss[oh],
                        mybir.ActivationFunctionType.Relu,
                    )
                    if i == nt - 1:
                        nc.scalar.dma_start(
                            out_h[bass.ts(i, P), bass.ts(oh, 512)],
                            out_sb[:, bass.ts(oh, 512)],
                        )
                if i != nt - 1:
                    nc.scalar.dma_start(out_h[bass.ts(i, P), :], out_sb)

            # 3-stage software pipeline: load+reduce @ i, PE accumulate @
            # i-1, transpose+GEMM @ i-2
            for i in range(nt + 2):
                if i < nt:
                    stage_load(i)
                    stage_reduce(i)
                if 1 <= i < nt + 1:
                    stage_pe(i - 1)
                if i >= 2:
                    stage_gemm(i - 2)

    nc.compile()
    return nc


def shard_inputs(inputs: dict) -> list[dict]:
    n = inputs["self_vecs"].shape[0]
    per = n // N_CORES
    # pack [neigh ; self] as [N, G, D] f32; quantize with error feedback
    # along the group axis: fp8 groups first (carrying the rounding
    # residual forward), bf16 groups last (absorbing the residual at
    # bf16 precision)
    x = np.empty((n, G, D), dtype=np.float32)
    x[:, :K] = inputs["neigh_vecs"]
    x[:, K] = inputs["self_vecs"]
    pk8 = np.empty((n, G8, D), dtype=NP_F8)
    pkb = np.empty((n, K_BF, D), dtype=NP_BF)
    e = np.zeros((n, D), dtype=np.float32)
    for g in range(G8):
        t = x[:, g] + e
        pk8[:, g] = t.astype(NP_F8)
        e = t - pk8[:, g].astype(np.float32)
    for g in range(K_BF):
        t = x[:, G8 + g] + e
        pkb[:, g] = t.astype(NP_BF)
        e = t - pkb[:, g].astype(np.float32)
    # pre-rearrange W to [p, c, o] so the device load is contiguous rows
    w_bf = np.ascontiguousarray(
        inputs["W"].reshape(D // P, P, O).transpose(1, 0, 2).reshape(P, -1),
        dtype=NP_BF,
    )
    pk8 = pk8.reshape(n, G8 * D)
    pkb = pkb.reshape(n, K_BF * D)
    maps = []
    for c in range(N_CORES):
        sl = slice(c * per, (c + 1) * per)
        maps.append({"pk8": pk8[sl], "pkb": pkb[sl], "W": w_bf})
    return maps


def run_sharded(inputs: dict, trace: bool = False, **kwargs):
    from concourse.bass_utils import run_bass_kernel_spmd

    in_maps = shard_inputs(inputs)
    n_nodes = in_maps[0]["pk8"].shape[0]
    nc = build_nc(n_nodes)
    res = run_bass_kernel_spmd(
        nc, in_maps, core_ids=list(range(N_CORES)), trace=trace, **kwargs
    )
    out = np.concatenate(
        [res.results[c]["out"] for c in range(N_CORES)], axis=0
    ).astype(np.float32)
    return out, res


def kernel(**inputs) -> np.ndarray:
    out, _ = run_sharded(inputs, trace=False)
    return out
